# revision 23
# baseline (speedup 1.0000x reference)
"""Trainium2 Bass kernel for nn_Block_85598698209846 (moe_routing).

Strategy (8 NeuronCores, SPMD single program, per-core data):
- Tokens are assigned to cores BY EXPERT (host routes via eids): core c owns
  exactly the tokens that route to expert c, sorted by (batch, position).
  MoE then needs no communication and each core loads only its expert.
- Attention: K/V are computed in contiguous position blocks (core r owns
  block r) and shared via one 8-core AllGather; each core computes Q for its
  scattered-but-sorted tokens. Causality is recovered with compile-time
  column windows (shared across cores) plus small per-core uploaded masks.
  Softmax runs without max-subtraction (|scores| <= 8 since q,k are
  RMS-normed and scaled by 1/8), matching the reference exactly.
- Layout: all activations transposed [D on partitions, tokens on free], so
  no on-device transposes anywhere; host pre-transposes weights/slices.
"""
import contextlib
import numpy as np

import concourse.bass as bass
import concourse.bacc as bacc
import concourse.tile as tile
from concourse import mybir
from concourse.bass_utils import run_bass_kernel_spmd

B, S, D = 2, 2048, 1024
NH, NKV, HD = 16, 4, 64
KVD = NKV * HD
NE, INTER = 8, 512
EPS = float(np.float32(1.1920929e-07))
NCORES = 8
KVBLK = 512          # seq rows per core in the KV phase
NKVT = S // 128      # 16 kv tiles per batch
NDT = D // 128       # 8 d-tiles
F32 = mybir.dt.float32
STRIP_MAX = 1536     # max score-strip width (3 PSUM banks)
ALU = mybir.AluOpType
ACT = mybir.ActivationFunctionType


# ---------------------------------------------------------------- host side

def _route(eids):
    eids = np.asarray(eids).astype(np.int64)
    lists = [[np.sort(np.where(eids[b] == e)[0]) for b in range(B)]
             for e in range(NE)]
    maxn = max(len(lists[e][b]) for e in range(NE) for b in range(B))
    CB = max(64, ((maxn + 63) // 64) * 64)
    cols = np.zeros((NE, B, CB), dtype=np.int64)
    nreal = np.zeros((NE, B), dtype=np.int64)
    for e in range(NE):
        for b in range(B):
            L = lists[e][b]
            nreal[e, b] = len(L)
            if len(L):
                cols[e, b, :len(L)] = L
                cols[e, b, len(L):] = L[-1]
    return cols, nreal, CB


def _windows(cols, CB):
    Wt = np.zeros((B, NKVT), dtype=np.int64)
    Mt = np.zeros((B, NKVT), dtype=np.int64)
    for b in range(B):
        for j in range(NKVT):
            Wt[b, j] = min(int(np.searchsorted(cols[e, b], 128 * j))
                           for e in range(NE))
            Mt[b, j] = max(int(np.searchsorted(cols[e, b], 128 * j + 127))
                           for e in range(NE))
    return Wt, Mt


def _strip_groups(Wt, CB):
    """Per batch: greedy-pack kv tiles into strip groups of width <= STRIP_MAX.
    groups[b] = list of groups; each group = list of (j, ofs_in_group, Nw)."""
    groups = []
    for b in range(B):
        gs, cur, ofs = [], [], 0
        for j in range(NKVT):
            Nw = int(CB - Wt[b, j])
            if Nw <= 0:
                continue
            if ofs + Nw > STRIP_MAX:
                gs.append(cur)
                cur, ofs = [], 0
            cur.append((j, ofs, Nw))
            ofs += Nw
        if cur:
            gs.append(cur)
        groups.append(gs)
    return groups


def _mask_layout(Wt, Mt, CB):
    ofs, total = {}, 0
    for b in range(B):
        for j in range(NKVT):
            if Wt[b, j] >= CB:
                continue
            mw = int(min(Mt[b, j], CB) - Wt[b, j])
            if mw <= 0:
                continue
            ofs[(b, j)] = (total, mw)
            total += mw
    return ofs, max(total, 1)


def _rope_tables(positions):
    """[128, n] cos2/sin2 for full-tile rope (2 heads/tile, swap32 form)."""
    inv_freq = (1.0 / 10000.0 ** (np.arange(0, HD, 2, dtype=np.float32) / HD)
                ).astype(np.float32)
    fr = np.outer(positions.astype(np.float32), inv_freq).astype(np.float32)
    c = np.cos(fr).astype(np.float32).T             # [32, n]
    s = np.sin(fr).astype(np.float32).T
    cos2 = np.concatenate([c, c, c, c], axis=0)
    sin2 = np.concatenate([s, -s, s, -s], axis=0)
    return np.ascontiguousarray(cos2), np.ascontiguousarray(sin2)


def _vec8(v):
    return np.ascontiguousarray(np.asarray(v, np.float32).reshape(NDT, 128).T)


def _build_core_inputs(c, x, x0, vel, rm0, rm1, attn_scale, mlp_scale, mu_c,
                       qg8, cq_wT, ck_wT, cv_wT, proj_wT, gate_up, down,
                       cols, CB, Wt, Mt, mofs, MW):
    f = np.float32
    pos = cols[c]                                     # [B, CB]
    bidx = np.repeat(np.arange(B), CB)
    sidx = pos.reshape(-1)
    b_kv, blk = c // 4, c % 4
    rows = slice(KVBLK * blk, KVBLK * blk + KVBLK)
    cosq2, sinq2 = _rope_tables(sidx)
    cosk2, sink2 = _rope_tables(np.arange(KVBLK * blk, KVBLK * blk + KVBLK))
    mask = np.zeros((128, MW), f)
    for (b, j), (o, mw) in mofs.items():
        W = Wt[b, j]
        kvp = np.arange(128 * j, 128 * j + 128)
        mask[:, o:o + mw] = (pos[b, None, W:W + mw] >= kvp[:, None])
    T = lambda a: np.ascontiguousarray(a.T.astype(f))
    return {
        "xqT": T(x[bidx, sidx]), "x0qT": T(x0[bidx, sidx]),
        "velqT": T(vel[bidx, sidx]),
        "xkvT": T(x[b_kv, rows]), "x0kvT": T(x0[b_kv, rows]),
        "cq_wT": cq_wT, "ck_wT": ck_wT, "cv_wT": cv_wT, "proj_wT": proj_wT,
        "gu": np.ascontiguousarray(gate_up[c].astype(f)),
        "dn": np.ascontiguousarray(down[c].astype(f)),
        "rm0v": _vec8(rm0), "rm1v": _vec8(rm1), "ascalev": _vec8(attn_scale),
        "mscalev": _vec8(mlp_scale), "mucv": _vec8(mu_c),
        "qg8T": np.ascontiguousarray(qg8.reshape(1, NH)),
        "cosq2": cosq2, "sinq2": sinq2, "cosk2": cosk2, "sink2": sink2,
        "maskcat": np.ascontiguousarray(mask),
    }


_PROG_CACHE = {}


def _prep(inputs):
    f = np.float32
    x = np.asarray(inputs["x"], f)
    x0 = np.asarray(inputs["x0"], f)
    vel = np.asarray(inputs["vel"], f)
    resid_mix = np.asarray(inputs["resid_mix"], f)
    mu_c = np.clip(np.asarray(inputs["mu"], f), f(0.5), f(1.5)).astype(f)
    qg8 = (np.asarray(inputs["q_gain"], f) * f(0.125)).astype(f)
    cq_wT = np.ascontiguousarray(np.asarray(inputs["cq_w"], f).T)
    ck_wT = np.ascontiguousarray(np.asarray(inputs["ck_w"], f).T)
    cv_wT = np.ascontiguousarray(np.asarray(inputs["cv_w"], f).T)
    proj_wT = np.ascontiguousarray(np.asarray(inputs["proj_w"], f).T)

    cols, nreal, CB = _route(inputs["eids"])
    Wt, Mt = _windows(cols, CB)
    groups = _strip_groups(Wt, CB)
    mofs, MW = _mask_layout(Wt, Mt, CB)
    meta = (cols, nreal, CB, Wt, Mt, groups, mofs, MW)
    in_maps = [
        _build_core_inputs(c, x, x0, vel, resid_mix[0], resid_mix[1],
                           np.asarray(inputs["attn_scale"], f),
                           np.asarray(inputs["mlp_scale"], f), mu_c, qg8,
                           cq_wT, ck_wT, cv_wT, proj_wT,
                           np.asarray(inputs["gate_up"], f),
                           np.asarray(inputs["down"], f),
                           cols, CB, Wt, Mt, mofs, MW)
        for c in range(NCORES)
    ]
    return meta, in_maps


def _assemble(results, meta):
    f = np.float32
    cols, nreal, CB = meta[0], meta[1], meta[2]
    x_out = np.zeros((B, S, D), f)
    v_out = np.zeros((B, S, D), f)
    for c in range(NCORES):
        xoT = results[c]["xoutT"]
        vnT = results[c]["vnT"]
        for b in range(B):
            n = int(nreal[c, b])
            if n == 0:
                continue
            sl = slice(b * CB, b * CB + n)
            x_out[b, cols[c, b, :n]] = xoT[:, sl].T
            v_out[b, cols[c, b, :n]] = vnT[:, sl].T
    return x_out, v_out


def get_program(meta):
    cols, nreal, CB, Wt, Mt, groups, mofs, MW = meta
    key = (CB, MW, tuple(Wt.reshape(-1)), tuple(Mt.reshape(-1)))
    if key not in _PROG_CACHE:
        _PROG_CACHE[key] = build_program(CB, Wt, Mt, groups, mofs, MW)
    return _PROG_CACHE[key]


def kernel(**inputs):
    meta, in_maps = _prep(inputs)
    nc = get_program(meta)
    res = run_bass_kernel_spmd(nc, in_maps, core_ids=list(range(NCORES)))
    return _assemble(res.results, meta)


# ------------------------------------------------------------- device side

def _chunks(n, limit=512):
    return [(s, min(limit, n - s)) for s in range(0, n, limit)]


def build_program(CB, Wt, Mt, groups, mofs, MW, n_devices=NCORES, dbg=False):
    C = B * CB
    nc = bacc.Bacc("TRN2", target_bir_lowering=False, debug=False,
                   num_devices=n_devices)
    dt = F32
    d_in = {}
    for name, shape in [
        ("xqT", [D, C]), ("x0qT", [D, C]), ("velqT", [D, C]),
        ("xkvT", [D, KVBLK]), ("x0kvT", [D, KVBLK]),
        ("cq_wT", [D, D]), ("ck_wT", [D, KVD]), ("cv_wT", [D, KVD]),
        ("proj_wT", [D, D]), ("gu", [D, 2 * INTER]), ("dn", [INTER, D]),
        ("rm0v", [128, NDT]), ("rm1v", [128, NDT]), ("ascalev", [128, NDT]),
        ("mscalev", [128, NDT]), ("mucv", [128, NDT]), ("qg8T", [1, NH]),
        ("cosq2", [128, C]), ("sinq2", [128, C]),
        ("cosk2", [128, KVBLK]), ("sink2", [128, KVBLK]),
        ("maskcat", [128, MW]),
    ]:
        d_in[name] = nc.dram_tensor(name, shape, dt, kind="ExternalInput")
    d_xout = nc.dram_tensor("xoutT", [D, C], dt, kind="ExternalOutput")
    d_vn = nc.dram_tensor("vnT", [D, C], dt, kind="ExternalOutput")
    d_dbg = {}
    if dbg:
        for name, shape in [("dbg_nk", [D, KVBLK]), ("dbg_agk", [2048, 512]),
                            ("dbg_agv", [4096, 256]), ("dbg_qro", [NH * 64, C]),
                            ("dbg_yall", [D, C]), ("dbg_x2", [D, C]),
                            ("dbg_mn", [D, C])]:
            d_dbg[name] = nc.dram_tensor(name, shape, dt,
                                         kind="ExternalOutput")

    with tile.TileContext(nc) as tc:
        _emit(tc, nc, d_in, d_xout, d_vn, CB, Wt, Mt, groups, mofs, d_dbg)
    nc.compile()
    return nc


def _emit(tc, nc, d_in, d_xout, d_vn, CB, Wt, Mt, groups, mofs, d_dbg={}):
    C = B * CB
    dt = F32
    sy, gp, ve, sc, pe = nc.sync, nc.gpsimd, nc.vector, nc.scalar, nc.tensor

    es = contextlib.ExitStack()
    cst = es.enter_context(tc.tile_pool(name="const", bufs=1))
    agd = es.enter_context(tc.tile_pool(name="agD", bufs=1, space="DRAM"))

    ones128 = cst.tile([128, 1], dt, tag="ones128")
    ve.memset(ones128[:], 1.0)
    ind64 = cst.tile([128, 2], dt, tag="ind64")
    ve.memset(ind64[:], 0.0)
    ve.memset(ind64[0:64, 0:1], 1.0)
    ve.memset(ind64[64:128, 1:2], 1.0)
    epsc = cst.tile([128, 1], dt, tag="epsc")
    ve.memset(epsc[:], EPS)
    vecs = {}
    for nm in ("rm0v", "rm1v", "ascalev", "mscalev", "mucv"):
        t = cst.tile([128, NDT], dt, tag=nm, name=nm)
        sy.dma_start(t[:], d_in[nm].ap())
        vecs[nm] = t
    qg8T = cst.tile([1, NH], dt, tag="qg8T")
    sy.dma_start(qg8T[:], d_in["qg8T"].ap())
    tbl = {}
    for nm, w in (("cosq2", C), ("sinq2", C), ("cosk2", KVBLK),
                  ("sink2", KVBLK)):
        t = cst.tile([128, w], dt, tag=nm, name=nm)
        sy.dma_start(t[:], d_in[nm].ap())
        tbl[nm] = t
    mask_sb = cst.tile([128, d_in["maskcat"].shape[1]], dt, tag="mask")
    sy.dma_start(mask_sb[:], d_in["maskcat"].ap())

    agk_in = agd.tile([KVD, KVBLK], dt, tag="agk_in")
    agv_in = agd.tile([KVBLK, KVD], dt, tag="agv_in")
    agk_out = agd.tile([NCORES * KVD, KVBLK], dt, addr_space="Shared",
                       tag="agk_out")
    agv_out = agd.tile([NCORES * KVBLK, KVD], dt, addr_space="Shared",
                       tag="agv_out")

    def rms_norm_T(pool, rot, pstmp, in_tiles, width, out_tag):
        outs = [pool.tile([128, width], dt, tag=f"{out_tag}{i}",
                          name=f"{out_tag}{i}") for i in range(NDT)]
        for (s, w) in _chunks(width):
            sqs = []
            for i in range(NDT):
                sq = rot.tile([128, w], dt, tag="nsq", name="nsq", bufs=3)
                ve.tensor_mul(sq[:], in_tiles[i][:, s:s + w],
                              in_tiles[i][:, s:s + w])
                sqs.append(sq)
            ssum = pstmp.tile([1, w], dt, tag="nps", name="nps", bufs=2)
            for i in range(NDT):
                pe.matmul(ssum[:], ones128[:], sqs[i][:],
                          start=(i == 0), stop=(i == NDT - 1))
            rt = rot.tile([1, w], dt, tag="nrt", name="nrt", bufs=2)
            sc.activation(rt[:], ssum[:], ACT.Sqrt, bias=epsc[0:1],
                          scale=1.0 / D)
            ve.reciprocal(rt[:], rt[:])
            bc = rot.tile([128, w], dt, tag="nbc", name="nbc", bufs=2)
            gp.partition_broadcast(bc[:], rt[0:1, :])
            for i in range(NDT):
                ve.tensor_mul(outs[i][:, s:s + w], in_tiles[i][:, s:s + w],
                              bc[:])
        return outs

    def head_norm(rot, pstmp, src_ap, width, gains, out_ap):
        """src [128, width] (2 heads) -> normalized out_ap (SBUF).
        gains: optional pair of [1,1] APs multiplied into the inverses."""
        for (s, w) in _chunks(width):
            sq = rot.tile([128, w], dt, tag="hsq", name="hsq", bufs=2)
            sc.activation(sq[:], src_ap[:, s:s + w], ACT.Square)
            for hh in range(2):
                hs = pstmp.tile([1, w], dt, tag="hps", name="hps", bufs=2)
                pe.matmul(hs[:], ind64[:, hh:hh + 1], sq[:],
                          start=True, stop=True)
                rt = rot.tile([1, w], dt, tag="hrt", name="hrt", bufs=2)
                sc.activation(rt[:], hs[:], ACT.Sqrt, bias=epsc[0:1],
                              scale=1.0 / HD)
                ve.reciprocal(rt[:], rt[:])
                if gains is not None:
                    ve.tensor_scalar_mul(rt[:], rt[:], gains[hh])
                # partition_broadcast only writes base-0 full tiles on HW
                bch = rot.tile([64, w], dt, tag="hbc", name="hbc", bufs=2)
                gp.partition_broadcast(bch[:], rt[0:1, :])
                ve.tensor_mul(out_ap[64 * hh:64 * (hh + 1), s:s + w],
                              src_ap[64 * hh:64 * (hh + 1), s:s + w], bch[:])

    def rope(rot, in_tile, cos2, sin2, width, out_tile=None, out_pair=None):
        sw = rot.tile([128, width], dt, tag="rsw", name="rsw", bufs=2)
        for base in (0, 64):
            gp.tensor_copy(out=sw[base:base + 32, :],
                           in_=in_tile[base + 32:base + 64, :])
            gp.tensor_copy(out=sw[base + 32:base + 64, :],
                           in_=in_tile[base:base + 32, :])
        a = rot.tile([128, width], dt, tag="ra", name="ra", bufs=2)
        ve.tensor_mul(a[:], in_tile[:], cos2[:, 0:width])
        ve.tensor_mul(sw[:], sw[:], sin2[:, 0:width])
        if out_pair is not None:
            ve.tensor_add(out_pair[0][:], a[0:64, :], sw[0:64, :])
            ve.tensor_add(out_pair[1][:], a[64:128, :], sw[64:128, :])
        else:
            ve.tensor_add(out_tile[:], a[:], sw[:])

    # ============================ Stage A: KV ============================
    with tc.tile_pool(name="kvA", bufs=1) as kva, \
         tc.tile_pool(name="kvR", bufs=2) as kvr, \
         tc.tile_pool(name="kvP", bufs=2, space="PSUM") as kvp:
        xm = []
        for i in range(NDT):
            xk = kvr.tile([128, KVBLK], dt, tag="xk", name="xk", bufs=2)
            sy.dma_start(xk[:], d_in["xkvT"].ap()[128 * i:128 * (i + 1), :])
            x0k = kvr.tile([128, KVBLK], dt, tag="x0k", name="x0k", bufs=2)
            sy.dma_start(x0k[:], d_in["x0kvT"].ap()[128 * i:128 * (i + 1), :])
            ve.tensor_scalar_mul(x0k[:], x0k[:], vecs["rm1v"][:, i:i + 1])
            t = kva.tile([128, KVBLK], dt, tag=f"xmk{i}", name=f"xmk{i}")
            ve.scalar_tensor_tensor(t[:], xk[:], vecs["rm0v"][:, i:i + 1],
                                    x0k[:], ALU.mult, ALU.add)
            xm.append(t)
        nk = rms_norm_T(kva, kvr, kvp, xm, KVBLK, "nk")
        if d_dbg:
            for i in range(NDT):
                sy.dma_start(d_dbg["dbg_nk"].ap()[128 * i:128 * (i + 1), :],
                             nk[i][:])
        ckw, cvw = [], []
        for i in range(NDT):
            t = kva.tile([128, KVD], dt, tag=f"ckw{i}", name=f"ckw{i}")
            sy.dma_start(t[:], d_in["ck_wT"].ap()[128 * i:128 * (i + 1), :])
            ckw.append(t)
            t2 = kva.tile([128, KVD], dt, tag=f"cvw{i}", name=f"cvw{i}")
            sy.dma_start(t2[:], d_in["cv_wT"].ap()[128 * i:128 * (i + 1), :])
            cvw.append(t2)
        for m in range(2):
            pkT = kvp.tile([128, KVBLK], dt, tag="pkT", name="pkT", bufs=2)
            for i in range(NDT):
                pe.matmul(pkT[:], ckw[i][:, 128 * m:128 * (m + 1)], nk[i][:],
                          start=(i == 0), stop=(i == NDT - 1))
            khat = kvr.tile([128, KVBLK], dt, tag="khat", name="khat", bufs=2)
            head_norm(kvr, kvp, pkT, KVBLK, None, khat)
            kro = kvr.tile([128, KVBLK], dt, tag="kro", name="kro", bufs=2)
            rope(kvr, khat, tbl["cosk2"], tbl["sink2"], KVBLK, kro)
            sy.dma_start(agk_in[128 * m:128 * (m + 1), :], kro[:])
        for m in range(4):
            pv = kvp.tile([128, KVD], dt, tag="pv", name="pv", bufs=2)
            for i in range(NDT):
                pe.matmul(pv[:], nk[i][:, 128 * m:128 * (m + 1)], cvw[i][:],
                          start=(i == 0), stop=(i == NDT - 1))
            vsb = kvr.tile([128, KVD], dt, tag="vsb", name="vsb", bufs=2)
            ve.tensor_copy(vsb[:], pv[:])
            sy.dma_start(agv_in[128 * m:128 * (m + 1), :], vsb[:])

    gp.collective_compute("AllGather", ALU.bypass,
                          replica_groups=[list(range(NCORES))],
                          ins=[agk_in.opt()], outs=[agk_out.opt()])
    gp.collective_compute("AllGather", ALU.bypass,
                          replica_groups=[list(range(NCORES))],
                          ins=[agv_in.opt()], outs=[agv_out.opt()])
    if d_dbg:
        sy.dma_start(d_dbg["dbg_agk"].ap(), agk_out[:, :])
        sy.dma_start(d_dbg["dbg_agv"].ap(), agv_out[:, :])

    # ===================== Stage B1: Q mix/norm/proj/rope ====================
    qa = es.enter_context(tc.tile_pool(name="qa", bufs=1))      # xmq: ->B4
    yap = es.enter_context(tc.tile_pool(name="yap", bufs=1))    # yall: ->B3
    xmq = [qa.tile([128, C], dt, tag=f"xmq{i}", name=f"xmq{i}")
           for i in range(NDT)]
    yall = [yap.tile([128, C], dt, tag=f"yall{i}", name=f"yall{i}")
            for i in range(NDT)]
    with tc.tile_pool(name="qrop", bufs=1) as qrp:
        qro = [qrp.tile([64, C], dt, tag=f"qro{h}", name=f"qro{h}")
               for h in range(NH)]
        with tc.tile_pool(name="qt", bufs=1) as qt, \
             tc.tile_pool(name="qrot", bufs=2) as qr2, \
             tc.tile_pool(name="qP", bufs=2, space="PSUM") as qp:
            for i in range(NDT):
                xq = qr2.tile([128, C], dt, tag="xq", name="xq", bufs=2)
                sy.dma_start(xq[:], d_in["xqT"].ap()[128 * i:128 * (i + 1), :])
                x0q = qr2.tile([128, C], dt, tag="x0q", name="x0q", bufs=2)
                sy.dma_start(x0q[:],
                             d_in["x0qT"].ap()[128 * i:128 * (i + 1), :])
                ve.tensor_scalar_mul(x0q[:], x0q[:], vecs["rm1v"][:, i:i + 1])
                ve.scalar_tensor_tensor(xmq[i][:], xq[:],
                                        vecs["rm0v"][:, i:i + 1],
                                        x0q[:], ALU.mult, ALU.add)
            nq = rms_norm_T(qt, qr2, qp, xmq, C, "nq")
            # q^T = cq_w @ n^T, streamed in column halves of cq_wT
            for half in range(2):
                cqh = []
                for i in range(NDT):
                    t = qt.tile([128, 512], dt, tag=f"cqh{i}",
                                name=f"cqh{i}")
                    sy.dma_start(t[:], d_in["cq_wT"].ap()
                                 [128 * i:128 * (i + 1),
                                  512 * half:512 * (half + 1)])
                    cqh.append(t)
                for mm in range(4):
                    m = 4 * half + mm
                    qhat = qr2.tile([128, C], dt, tag="qhat", name="qhat",
                                    bufs=2)
                    for (s, w) in _chunks(C):
                        psq = qp.tile([128, w], dt, tag="psq", name="psq",
                                      bufs=2)
                        for i in range(NDT):
                            pe.matmul(psq[:],
                                      cqh[i][:, 128 * mm:128 * (mm + 1)],
                                      nq[i][:, s:s + w],
                                      start=(i == 0), stop=(i == NDT - 1))
                        head_norm(qr2, qp, psq, w,
                                  (qg8T[0:1, 2 * m:2 * m + 1],
                                   qg8T[0:1, 2 * m + 1:2 * m + 2]),
                                  qhat[:, s:s + w])
                    rope(qr2, qhat, tbl["cosq2"], tbl["sinq2"], C,
                         out_pair=(qro[2 * m], qro[2 * m + 1]))

        if d_dbg:
            for h in range(NH):
                sy.dma_start(d_dbg["dbg_qro"].ap()[64 * h:64 * (h + 1), :],
                             qro[h][:])
        # ========================= Stage B2: attention =========================
        with tc.tile_pool(name="at", bufs=1) as at, \
             tc.tile_pool(name="atP", bufs=2, space="PSUM") as atp:
            for b in range(B):
                all_js = [j for g in groups[b] for (j, _, _) in g]
                for kh in range(NKV):
                    kts, vexts = {}, {}
                    for g in groups[b]:
                        for (j, _, _) in g:
                            r = 4 * b + j // 4
                            loc = 128 * (j % 4)
                            kt = at.tile([64, 128], dt, tag="kt", name="kt",
                                         bufs=34)
                            sy.dma_start(
                                kt[:],
                                agk_out[KVD * r + 64 * kh:
                                        KVD * r + 64 * (kh + 1),
                                        loc:loc + 128])
                            kts[j] = kt
                            vx = at.tile([128, 65], dt, tag="vx", name="vx",
                                         bufs=34)
                            sy.dma_start(
                                vx[:, 0:64],
                                agv_out[KVBLK * r + loc:KVBLK * r + loc + 128,
                                        64 * kh:64 * (kh + 1)])
                            gp.memset(vx[:, 64:65], 1.0)
                            vexts[j] = vx
                    for hp in range(2):
                        h0 = 4 * kh + 2 * hp
                        pys = [atp.tile([65, CB], dt, tag="py", name="py",
                                        bufs=2) for _ in range(2)]
                        for g in groups[b]:
                            gw = g[-1][1] + g[-1][2]
                            sts = [atp.tile([128, gw], dt, tag="st",
                                            name="st", bufs=2,
                                            padded_shape=[128, STRIP_MAX])
                                   for _ in range(2)]
                            prb = at.tile([128, 2 * gw], dt, tag="prb",
                                          name="prb", bufs=2,
                                          padded_shape=[128, 2 * STRIP_MAX])
                            for (j, ofs, Nw) in g:
                                W = int(Wt[b, j])
                                for hh in range(2):
                                    qs = qro[h0 + hh][:,
                                             b * CB + W:b * CB + CB]
                                    p0 = 0
                                    while p0 < Nw:
                                        bend = ((ofs + p0) // 512 + 1) * 512
                                        pw = min(Nw - p0, bend - (ofs + p0))
                                        pe.matmul(
                                            sts[hh][:, ofs + p0:ofs + p0 + pw],
                                            kts[j][:], qs[:, p0:p0 + pw],
                                            start=True, stop=True)
                                        p0 += pw
                            for hh in range(2):
                                sc.activation(prb[:, gw * hh:gw * (hh + 1)],
                                              sts[hh][:], ACT.Exp)
                            for (j, ofs, Nw) in g:
                                if (b, j) not in mofs:
                                    continue
                                mo, mw = mofs[(b, j)]
                                mw = min(mw, Nw)
                                for hh in range(2):
                                    o2 = gw * hh + ofs
                                    ve.tensor_mul(prb[:, o2:o2 + mw],
                                                  prb[:, o2:o2 + mw],
                                                  mask_sb[:, mo:mo + mw])
                            for (j, ofs, Nw) in g:
                                W = int(Wt[b, j])
                                for hh in range(2):
                                    pe.matmul(
                                        pys[hh][:, W:CB], vexts[j][:],
                                        prb[:, gw * hh + ofs:
                                            gw * hh + ofs + Nw],
                                        start=(j == all_js[0]),
                                        stop=(j == all_js[-1]),
                                        skip_group_check=True)
                        for hh in range(2):
                            h = h0 + hh
                            rc = at.tile([1, CB], dt, tag="rc", name="rc",
                                         bufs=2)
                            ve.reciprocal(rc[:], pys[hh][64:65, :])
                            yb = at.tile([64, CB], dt, tag="yb", name="yb",
                                         bufs=2)
                            gp.partition_broadcast(yb[:], rc[0:1, :])
                            ve.tensor_mul(
                                yall[h // 2][64 * (h % 2):64 * (h % 2) + 64,
                                             b * CB:b * CB + CB],
                                pys[hh][0:64, :], yb[:])

    if d_dbg:
        for i in range(NDT):
            sy.dma_start(d_dbg["dbg_yall"].ap()[128 * i:128 * (i + 1), :],
                         yall[i][:])
    # ===================== Stage B3: out-proj + PID =====================
    with tc.tile_pool(name="pj", bufs=1) as pj, \
         tc.tile_pool(name="pjR", bufs=2) as pjr, \
         tc.tile_pool(name="pjP", bufs=2, space="PSUM") as pjp:
        for half in range(2):
            pjh = []
            for i in range(NDT):
                t = pj.tile([128, 512], dt, tag=f"pjh{i}", name=f"pjh{i}")
                sy.dma_start(t[:], d_in["proj_wT"].ap()
                             [128 * i:128 * (i + 1),
                              512 * half:512 * (half + 1)])
                pjh.append(t)
            for mm in range(4):
                m = 4 * half + mm
                velm = pjr.tile([128, C], dt, tag="velm", name="velm",
                                bufs=2)
                sy.dma_start(velm[:],
                             d_in["velqT"].ap()[128 * m:128 * (m + 1), :])
                for (s, w) in _chunks(C):
                    pso = pjp.tile([128, w], dt, tag="pso", name="pso",
                                   bufs=2)
                    for i in range(NDT):
                        pe.matmul(pso[:], pjh[i][:, 128 * mm:128 * (mm + 1)],
                                  yall[i][:, s:s + w],
                                  start=(i == 0), stop=(i == NDT - 1))
                    ve.scalar_tensor_tensor(
                        xmq[m][:, s:s + w], pso[:],
                        vecs["ascalev"][:, m:m + 1],
                        xmq[m][:, s:s + w], ALU.mult, ALU.add)
                t2 = pjr.tile([128, C], dt, tag="t2", name="t2", bufs=2)
                ve.tensor_scalar(t2[:], xmq[m][:], vecs["mucv"][:, m:m + 1],
                                 0.3, ALU.subtract, ALU.mult)
                vn = pjr.tile([128, C], dt, tag="vn", name="vn", bufs=2)
                ve.scalar_tensor_tensor(vn[:], velm[:], 0.95, t2[:],
                                        ALU.mult, ALU.subtract)
                ve.tensor_scalar(vn[:], vn[:], 3.0, -3.0, ALU.min, ALU.max)
                sy.dma_start(d_vn.ap()[128 * m:128 * (m + 1), :], vn[:])
                ve.scalar_tensor_tensor(xmq[m][:], vn[:], 0.1 * 0.1,
                                        xmq[m][:], ALU.mult, ALU.add)

    # ============================ Stage B4: MoE ============================
    with tc.tile_pool(name="mo", bufs=1) as mo, \
         tc.tile_pool(name="moR", bufs=2) as mor, \
         tc.tile_pool(name="moP", bufs=2, space="PSUM") as mop:
        if d_dbg:
            for i in range(NDT):
                sy.dma_start(d_dbg["dbg_x2"].ap()[128 * i:128 * (i + 1), :],
                             xmq[i][:])
        mn = rms_norm_T(mo, mor, mop, xmq, C, "mn")
        if d_dbg:
            for i in range(NDT):
                sy.dma_start(d_dbg["dbg_mn"].ap()[128 * i:128 * (i + 1), :],
                             mn[i][:])
        sg, hh_t = [], []
        for half in range(2):
            guh = []
            for i in range(NDT):
                t = mo.tile([128, 512], dt, tag=f"guh{i}", name=f"guh{i}",
                            bufs=2)
                sy.dma_start(t[:], d_in["gu"].ap()
                             [128 * i:128 * (i + 1),
                              512 * half:512 * (half + 1)])
                guh.append(t)
            for mm in range(4):
                m = 4 * half + mm
                for (s, w) in _chunks(C):
                    psh = mop.tile([128, w], dt, tag="psh", name="psh",
                                   bufs=2)
                    for i in range(NDT):
                        pe.matmul(psh[:], guh[i][:, 128 * mm:128 * (mm + 1)],
                                  mn[i][:, s:s + w],
                                  start=(i == 0), stop=(i == NDT - 1))
                    if m < 4:
                        if s == 0:
                            sgm = mo.tile([128, C], dt, tag=f"sg{m}",
                                          name=f"sg{m}")
                            sg.append(sgm)
                        # silu(g) = g * sigmoid(g)
                        sc.activation(sg[m][:, s:s + w], psh[:], ACT.Sigmoid)
                        ve.tensor_mul(sg[m][:, s:s + w], sg[m][:, s:s + w],
                                      psh[:])
                    else:
                        if s == 0:
                            hm = mo.tile([128, C], dt, tag=f"hh{m - 4}",
                                         name=f"hh{m - 4}")
                            hh_t.append(hm)
                        ve.tensor_mul(hh_t[m - 4][:, s:s + w],
                                      sg[m - 4][:, s:s + w], psh[:])
        dnw = []
        for i2 in range(4):
            t = mo.tile([128, D], dt, tag=f"dnw{i2}", name=f"dnw{i2}")
            sy.dma_start(t[:], d_in["dn"].ap()[128 * i2:128 * (i2 + 1), :])
            dnw.append(t)
        for m in range(NDT):
            xo = mor.tile([128, C], dt, tag="xo", name="xo", bufs=2)
            for (s, w) in _chunks(C):
                psm = mop.tile([128, w], dt, tag="psm", name="psm", bufs=2)
                for i2 in range(4):
                    pe.matmul(psm[:], dnw[i2][:, 128 * m:128 * (m + 1)],
                              hh_t[i2][:, s:s + w],
                              start=(i2 == 0), stop=(i2 == 3))
                ve.scalar_tensor_tensor(xo[:, s:s + w], psm[:],
                                        vecs["mscalev"][:, m:m + 1],
                                        xmq[m][:, s:s + w],
                                        ALU.mult, ALU.add)
            sy.dma_start(d_xout.ap()[128 * m:128 * (m + 1), :], xo[:])

    es.close()


# revision 24
# speedup vs baseline: 1.4052x; 1.4052x over previous
"""Trainium2 Bass kernel for nn_Block_85598698209846 (moe_routing).

Strategy (8 NeuronCores, SPMD single program, per-core data):
- Tokens are assigned to cores BY EXPERT (host routes via eids): core c owns
  exactly the tokens that route to expert c, sorted by (batch, position).
  MoE then needs no communication and each core loads only its expert.
- Attention: K/V are computed in contiguous position blocks (core r owns
  block r) and shared via one 8-core AllGather; each core computes Q for its
  scattered-but-sorted tokens. Causality is recovered with compile-time
  column windows (shared across cores) plus small per-core uploaded masks.
  Softmax runs without max-subtraction (|scores| <= 8 since q,k are
  RMS-normed and scaled by 1/8), matching the reference exactly.
- Layout: all activations transposed [D on partitions, tokens on free], so
  no on-device transposes anywhere; host pre-transposes weights/slices.
"""
import contextlib
import numpy as np
import ml_dtypes

import concourse.bass as bass
import concourse.bacc as bacc
import concourse.tile as tile
from concourse import mybir
from concourse.bass_utils import run_bass_kernel_spmd

B, S, D = 2, 2048, 1024
NH, NKV, HD = 16, 4, 64
KVD = NKV * HD
NE, INTER = 8, 512
EPS = float(np.float32(1.1920929e-07))
NCORES = 8
KVBLK = 512          # seq rows per core in the KV phase
NKVT = S // 128      # 16 kv tiles per batch
NDT = D // 128       # 8 d-tiles
F32 = mybir.dt.float32
BF16 = mybir.dt.bfloat16
STRIP_MAX = 1536     # max score-strip width (3 PSUM banks)
ALU = mybir.AluOpType
ACT = mybir.ActivationFunctionType


# ---------------------------------------------------------------- host side

def _route(eids):
    eids = np.asarray(eids).astype(np.int64)
    lists = [[np.sort(np.where(eids[b] == e)[0]) for b in range(B)]
             for e in range(NE)]
    maxn = max(len(lists[e][b]) for e in range(NE) for b in range(B))
    CB = max(64, ((maxn + 63) // 64) * 64)
    cols = np.zeros((NE, B, CB), dtype=np.int64)
    nreal = np.zeros((NE, B), dtype=np.int64)
    for e in range(NE):
        for b in range(B):
            L = lists[e][b]
            nreal[e, b] = len(L)
            if len(L):
                cols[e, b, :len(L)] = L
                cols[e, b, len(L):] = L[-1]
    return cols, nreal, CB


def _windows(cols, CB):
    Wt = np.zeros((B, NKVT), dtype=np.int64)
    Mt = np.zeros((B, NKVT), dtype=np.int64)
    for b in range(B):
        for j in range(NKVT):
            Wt[b, j] = min(int(np.searchsorted(cols[e, b], 128 * j))
                           for e in range(NE))
            Mt[b, j] = max(int(np.searchsorted(cols[e, b], 128 * j + 127))
                           for e in range(NE))
    return Wt, Mt


def _strip_groups(Wt, CB):
    """Per batch: greedy-pack kv tiles into strip groups of width <= STRIP_MAX.
    groups[b] = list of groups; each group = list of (j, ofs_in_group, Nw)."""
    groups = []
    for b in range(B):
        gs, cur, ofs = [], [], 0
        for j in range(NKVT):
            Nw = int(CB - Wt[b, j])
            if Nw <= 0:
                continue
            if ofs + Nw > STRIP_MAX:
                gs.append(cur)
                cur, ofs = [], 0
            cur.append((j, ofs, Nw))
            ofs += Nw
        if cur:
            gs.append(cur)
        groups.append(gs)
    return groups


def _mask_layout(Wt, Mt, CB):
    ofs, total = {}, 0
    for b in range(B):
        for j in range(NKVT):
            if Wt[b, j] >= CB:
                continue
            mw = int(min(Mt[b, j], CB) - Wt[b, j])
            if mw <= 0:
                continue
            ofs[(b, j)] = (total, mw)
            total += mw
    return ofs, max(total, 1)


def _rope_tables(positions):
    """[128, n] cos2/sin2 for full-tile rope (2 heads/tile, swap32 form)."""
    inv_freq = (1.0 / 10000.0 ** (np.arange(0, HD, 2, dtype=np.float32) / HD)
                ).astype(np.float32)
    fr = np.outer(positions.astype(np.float32), inv_freq).astype(np.float32)
    c = np.cos(fr).astype(np.float32).T             # [32, n]
    s = np.sin(fr).astype(np.float32).T
    cos2 = np.concatenate([c, c, c, c], axis=0)
    sin2 = np.concatenate([s, -s, s, -s], axis=0)
    return np.ascontiguousarray(cos2), np.ascontiguousarray(sin2)


def _vec8(v):
    return np.ascontiguousarray(np.asarray(v, np.float32).reshape(NDT, 128).T)


def _build_core_inputs(c, x, x0, vel, rm0, rm1, attn_scale, mlp_scale, mu_c,
                       qg8, cq_wT, ck_wT, cv_wT, proj_wT, gate_up, down,
                       cols, CB, Wt, Mt, mofs, MW):
    f = np.float32
    pos = cols[c]                                     # [B, CB]
    bidx = np.repeat(np.arange(B), CB)
    sidx = pos.reshape(-1)
    b_kv, blk = c // 4, c % 4
    rows = slice(KVBLK * blk, KVBLK * blk + KVBLK)
    cosq2, sinq2 = _rope_tables(sidx)
    cosk2, sink2 = _rope_tables(np.arange(KVBLK * blk, KVBLK * blk + KVBLK))
    mask = np.zeros((128, MW), f)
    for (b, j), (o, mw) in mofs.items():
        W = Wt[b, j]
        kvp = np.arange(128 * j, 128 * j + 128)
        mask[:, o:o + mw] = (pos[b, None, W:W + mw] >= kvp[:, None])
    T = lambda a: np.ascontiguousarray(a.T.astype(f))
    return {
        "xqT": T(x[bidx, sidx]), "x0qT": T(x0[bidx, sidx]),
        "velqT": T(vel[bidx, sidx]),
        "xkvT": T(x[b_kv, rows]), "x0kvT": T(x0[b_kv, rows]),
        "cq_wT": cq_wT, "ck_wT": ck_wT, "cv_wT": cv_wT, "proj_wT": proj_wT,
        "gu": np.ascontiguousarray(gate_up[c].astype(f)),
        "dn": np.ascontiguousarray(down[c].astype(f)),
        "rm0v": _vec8(rm0), "rm1v": _vec8(rm1), "ascalev": _vec8(attn_scale),
        "mscalev": _vec8(mlp_scale), "mucv": _vec8(mu_c),
        "qg8T": np.ascontiguousarray(qg8.reshape(1, NH)),
        "cosq2": cosq2, "sinq2": sinq2, "cosk2": cosk2, "sink2": sink2,
        "maskcat": np.ascontiguousarray(mask.astype(ml_dtypes.bfloat16)),
    }


_PROG_CACHE = {}


def _prep(inputs):
    f = np.float32
    x = np.asarray(inputs["x"], f)
    x0 = np.asarray(inputs["x0"], f)
    vel = np.asarray(inputs["vel"], f)
    resid_mix = np.asarray(inputs["resid_mix"], f)
    mu_c = np.clip(np.asarray(inputs["mu"], f), f(0.5), f(1.5)).astype(f)
    qg8 = (np.asarray(inputs["q_gain"], f) * f(0.125)).astype(f)
    cq_wT = np.ascontiguousarray(np.asarray(inputs["cq_w"], f).T)
    ck_wT = np.ascontiguousarray(np.asarray(inputs["ck_w"], f).T)
    cv_wT = np.ascontiguousarray(np.asarray(inputs["cv_w"], f).T)
    proj_wT = np.ascontiguousarray(np.asarray(inputs["proj_w"], f).T)

    cols, nreal, CB = _route(inputs["eids"])
    Wt, Mt = _windows(cols, CB)
    groups = _strip_groups(Wt, CB)
    mofs, MW = _mask_layout(Wt, Mt, CB)
    meta = (cols, nreal, CB, Wt, Mt, groups, mofs, MW)
    in_maps = [
        _build_core_inputs(c, x, x0, vel, resid_mix[0], resid_mix[1],
                           np.asarray(inputs["attn_scale"], f),
                           np.asarray(inputs["mlp_scale"], f), mu_c, qg8,
                           cq_wT, ck_wT, cv_wT, proj_wT,
                           np.asarray(inputs["gate_up"], f),
                           np.asarray(inputs["down"], f),
                           cols, CB, Wt, Mt, mofs, MW)
        for c in range(NCORES)
    ]
    return meta, in_maps


def _assemble(results, meta):
    f = np.float32
    cols, nreal, CB = meta[0], meta[1], meta[2]
    x_out = np.zeros((B, S, D), f)
    v_out = np.zeros((B, S, D), f)
    for c in range(NCORES):
        xoT = results[c]["xoutT"]
        vnT = results[c]["vnT"]
        for b in range(B):
            n = int(nreal[c, b])
            if n == 0:
                continue
            sl = slice(b * CB, b * CB + n)
            x_out[b, cols[c, b, :n]] = xoT[:, sl].T
            v_out[b, cols[c, b, :n]] = vnT[:, sl].T
    return x_out, v_out


def get_program(meta):
    cols, nreal, CB, Wt, Mt, groups, mofs, MW = meta
    key = (CB, MW, tuple(Wt.reshape(-1)), tuple(Mt.reshape(-1)))
    if key not in _PROG_CACHE:
        _PROG_CACHE[key] = build_program(CB, Wt, Mt, groups, mofs, MW)
    return _PROG_CACHE[key]


def kernel(**inputs):
    meta, in_maps = _prep(inputs)
    nc = get_program(meta)
    res = run_bass_kernel_spmd(nc, in_maps, core_ids=list(range(NCORES)))
    return _assemble(res.results, meta)


# ------------------------------------------------------------- device side

def _chunks(n, limit=512):
    return [(s, min(limit, n - s)) for s in range(0, n, limit)]


def build_program(CB, Wt, Mt, groups, mofs, MW, n_devices=NCORES, dbg=False):
    C = B * CB
    nc = bacc.Bacc("TRN2", target_bir_lowering=False, debug=False,
                   num_devices=n_devices)
    dt = F32
    d_in = {}
    for name, shape in [
        ("xqT", [D, C]), ("x0qT", [D, C]), ("velqT", [D, C]),
        ("xkvT", [D, KVBLK]), ("x0kvT", [D, KVBLK]),
        ("cq_wT", [D, D]), ("ck_wT", [D, KVD]), ("cv_wT", [D, KVD]),
        ("proj_wT", [D, D]), ("gu", [D, 2 * INTER]), ("dn", [INTER, D]),
        ("rm0v", [128, NDT]), ("rm1v", [128, NDT]), ("ascalev", [128, NDT]),
        ("mscalev", [128, NDT]), ("mucv", [128, NDT]), ("qg8T", [1, NH]),
        ("cosq2", [128, C]), ("sinq2", [128, C]),
        ("cosk2", [128, KVBLK]), ("sink2", [128, KVBLK]),
    ]:
        d_in[name] = nc.dram_tensor(name, shape, dt, kind="ExternalInput")
    d_in["maskcat"] = nc.dram_tensor("maskcat", [128, MW], BF16,
                                     kind="ExternalInput")
    d_xout = nc.dram_tensor("xoutT", [D, C], dt, kind="ExternalOutput")
    d_vn = nc.dram_tensor("vnT", [D, C], dt, kind="ExternalOutput")
    d_dbg = {}
    if dbg:
        for name, shape in [("dbg_nk", [D, KVBLK]), ("dbg_agk", [2048, 512]),
                            ("dbg_agv", [4096, 256]), ("dbg_qro", [NH * 64, C]),
                            ("dbg_yall", [D, C]), ("dbg_x2", [D, C]),
                            ("dbg_mn", [D, C])]:
            d_dbg[name] = nc.dram_tensor(name, shape, dt,
                                         kind="ExternalOutput")

    with tile.TileContext(nc) as tc:
        _emit(tc, nc, d_in, d_xout, d_vn, CB, Wt, Mt, groups, mofs, d_dbg)
    nc.compile()
    return nc


def _emit(tc, nc, d_in, d_xout, d_vn, CB, Wt, Mt, groups, mofs, d_dbg={}):
    C = B * CB
    dt = F32
    sy, gp, ve, sc, pe = nc.sync, nc.gpsimd, nc.vector, nc.scalar, nc.tensor

    es = contextlib.ExitStack()
    cst = es.enter_context(tc.tile_pool(name="const", bufs=1))
    agd = es.enter_context(tc.tile_pool(name="agD", bufs=1, space="DRAM"))

    ones128 = cst.tile([128, 1], dt, tag="ones128")
    ve.memset(ones128[:], 1.0)
    ind64 = cst.tile([128, 2], dt, tag="ind64")
    ve.memset(ind64[:], 0.0)
    ve.memset(ind64[0:64, 0:1], 1.0)
    ve.memset(ind64[64:128, 1:2], 1.0)
    epsc = cst.tile([128, 1], dt, tag="epsc")
    ve.memset(epsc[:], EPS)
    vecs = {}
    for nm in ("rm0v", "rm1v", "ascalev", "mscalev", "mucv"):
        t = cst.tile([128, NDT], dt, tag=nm, name=nm)
        sy.dma_start(t[:], d_in[nm].ap())
        vecs[nm] = t
    qg8T = cst.tile([1, NH], dt, tag="qg8T")
    sy.dma_start(qg8T[:], d_in["qg8T"].ap())
    tbl = {}
    for nm, w in (("cosq2", C), ("sinq2", C), ("cosk2", KVBLK),
                  ("sink2", KVBLK)):
        t = cst.tile([128, w], dt, tag=nm, name=nm)
        sy.dma_start(t[:], d_in[nm].ap())
        tbl[nm] = t
    mask_sb = cst.tile([128, d_in["maskcat"].shape[1]], BF16, tag="mask")
    sy.dma_start(mask_sb[:], d_in["maskcat"].ap())

    agk_in = agd.tile([KVD, KVBLK], BF16, tag="agk_in")
    agv_in = agd.tile([KVBLK, KVD], BF16, tag="agv_in")
    agk_out = agd.tile([NCORES * KVD, KVBLK], BF16, addr_space="Shared",
                       tag="agk_out")
    agv_out = agd.tile([NCORES * KVBLK, KVD], BF16, addr_space="Shared",
                       tag="agv_out")

    def rms_norm_T(pool, rot, pstmp, in_tiles, width, out_tag):
        outs = [pool.tile([128, width], dt, tag=f"{out_tag}{i}",
                          name=f"{out_tag}{i}") for i in range(NDT)]
        for (s, w) in _chunks(width):
            sqs = []
            for i in range(NDT):
                sq = rot.tile([128, w], dt, tag="nsq", name="nsq", bufs=3)
                ve.tensor_mul(sq[:], in_tiles[i][:, s:s + w],
                              in_tiles[i][:, s:s + w])
                sqs.append(sq)
            ssum = pstmp.tile([1, w], dt, tag="nps", name="nps", bufs=2)
            for i in range(NDT):
                pe.matmul(ssum[:], ones128[:], sqs[i][:],
                          start=(i == 0), stop=(i == NDT - 1))
            rt = rot.tile([1, w], dt, tag="nrt", name="nrt", bufs=2)
            sc.activation(rt[:], ssum[:], ACT.Sqrt, bias=epsc[0:1],
                          scale=1.0 / D)
            ve.reciprocal(rt[:], rt[:])
            bc = rot.tile([128, w], dt, tag="nbc", name="nbc", bufs=2)
            gp.partition_broadcast(bc[:], rt[0:1, :])
            for i in range(NDT):
                ve.tensor_mul(outs[i][:, s:s + w], in_tiles[i][:, s:s + w],
                              bc[:])
        return outs

    def head_norm(rot, pstmp, src_ap, width, gains, out_ap):
        """src [128, width] (2 heads) -> normalized out_ap (SBUF).
        gains: optional pair of [1,1] APs multiplied into the inverses."""
        for (s, w) in _chunks(width):
            sq = rot.tile([128, w], dt, tag="hsq", name="hsq", bufs=2)
            sc.activation(sq[:], src_ap[:, s:s + w], ACT.Square)
            for hh in range(2):
                hs = pstmp.tile([1, w], dt, tag="hps", name="hps", bufs=2)
                pe.matmul(hs[:], ind64[:, hh:hh + 1], sq[:],
                          start=True, stop=True)
                rt = rot.tile([1, w], dt, tag="hrt", name="hrt", bufs=2)
                sc.activation(rt[:], hs[:], ACT.Sqrt, bias=epsc[0:1],
                              scale=1.0 / HD)
                ve.reciprocal(rt[:], rt[:])
                if gains is not None:
                    ve.tensor_scalar_mul(rt[:], rt[:], gains[hh])
                # partition_broadcast only writes base-0 full tiles on HW
                bch = rot.tile([64, w], dt, tag="hbc", name="hbc", bufs=2)
                gp.partition_broadcast(bch[:], rt[0:1, :])
                ve.tensor_mul(out_ap[64 * hh:64 * (hh + 1), s:s + w],
                              src_ap[64 * hh:64 * (hh + 1), s:s + w], bch[:])

    def rope(rot, in_tile, cos2, sin2, width, out_tile=None, out_pair=None):
        sw = rot.tile([128, width], dt, tag="rsw", name="rsw", bufs=2)
        for base in (0, 64):
            gp.tensor_copy(out=sw[base:base + 32, :],
                           in_=in_tile[base + 32:base + 64, :])
            gp.tensor_copy(out=sw[base + 32:base + 64, :],
                           in_=in_tile[base:base + 32, :])
        a = rot.tile([128, width], dt, tag="ra", name="ra", bufs=2)
        ve.tensor_mul(a[:], in_tile[:], cos2[:, 0:width])
        ve.tensor_mul(sw[:], sw[:], sin2[:, 0:width])
        if out_pair is not None:
            ve.tensor_add(out_pair[0][:], a[0:64, :], sw[0:64, :])
            ve.tensor_add(out_pair[1][:], a[64:128, :], sw[64:128, :])
        else:
            ve.tensor_add(out_tile[:], a[:], sw[:])

    # ============================ Stage A: KV ============================
    with tc.tile_pool(name="kvA", bufs=1) as kva, \
         tc.tile_pool(name="kvR", bufs=2) as kvr, \
         tc.tile_pool(name="kvP", bufs=2, space="PSUM") as kvp:
        xm = []
        for i in range(NDT):
            xk = kvr.tile([128, KVBLK], dt, tag="xk", name="xk", bufs=2)
            sy.dma_start(xk[:], d_in["xkvT"].ap()[128 * i:128 * (i + 1), :])
            x0k = kvr.tile([128, KVBLK], dt, tag="x0k", name="x0k", bufs=2)
            sy.dma_start(x0k[:], d_in["x0kvT"].ap()[128 * i:128 * (i + 1), :])
            ve.tensor_scalar_mul(x0k[:], x0k[:], vecs["rm1v"][:, i:i + 1])
            t = kva.tile([128, KVBLK], dt, tag=f"xmk{i}", name=f"xmk{i}")
            ve.scalar_tensor_tensor(t[:], xk[:], vecs["rm0v"][:, i:i + 1],
                                    x0k[:], ALU.mult, ALU.add)
            xm.append(t)
        nk = rms_norm_T(kva, kvr, kvp, xm, KVBLK, "nk")
        if d_dbg:
            for i in range(NDT):
                sy.dma_start(d_dbg["dbg_nk"].ap()[128 * i:128 * (i + 1), :],
                             nk[i][:])
        ckw, cvw = [], []
        for i in range(NDT):
            t = kva.tile([128, KVD], dt, tag=f"ckw{i}", name=f"ckw{i}")
            sy.dma_start(t[:], d_in["ck_wT"].ap()[128 * i:128 * (i + 1), :])
            ckw.append(t)
            t2 = kva.tile([128, KVD], dt, tag=f"cvw{i}", name=f"cvw{i}")
            sy.dma_start(t2[:], d_in["cv_wT"].ap()[128 * i:128 * (i + 1), :])
            cvw.append(t2)
        for m in range(2):
            pkT = kvp.tile([128, KVBLK], dt, tag="pkT", name="pkT", bufs=2)
            for i in range(NDT):
                pe.matmul(pkT[:], ckw[i][:, 128 * m:128 * (m + 1)], nk[i][:],
                          start=(i == 0), stop=(i == NDT - 1))
            khat = kvr.tile([128, KVBLK], dt, tag="khat", name="khat", bufs=2)
            head_norm(kvr, kvp, pkT, KVBLK, None, khat)
            kro = kvr.tile([128, KVBLK], BF16, tag="kro", name="kro", bufs=2)
            rope(kvr, khat, tbl["cosk2"], tbl["sink2"], KVBLK, kro)
            sy.dma_start(agk_in[128 * m:128 * (m + 1), :], kro[:])
        for m in range(4):
            pv = kvp.tile([128, KVD], dt, tag="pv", name="pv", bufs=2)
            for i in range(NDT):
                pe.matmul(pv[:], nk[i][:, 128 * m:128 * (m + 1)], cvw[i][:],
                          start=(i == 0), stop=(i == NDT - 1))
            vsb = kvr.tile([128, KVD], BF16, tag="vsb", name="vsb", bufs=2)
            ve.tensor_copy(vsb[:], pv[:])
            sy.dma_start(agv_in[128 * m:128 * (m + 1), :], vsb[:])

    gp.collective_compute("AllGather", ALU.bypass,
                          replica_groups=[list(range(NCORES))],
                          ins=[agk_in.opt()], outs=[agk_out.opt()])
    gp.collective_compute("AllGather", ALU.bypass,
                          replica_groups=[list(range(NCORES))],
                          ins=[agv_in.opt()], outs=[agv_out.opt()])
    if d_dbg:
        sy.dma_start(d_dbg["dbg_agk"].ap(), agk_out[:, :])
        sy.dma_start(d_dbg["dbg_agv"].ap(), agv_out[:, :])

    # ===================== Stage B1: Q mix/norm/proj/rope ====================
    qa = es.enter_context(tc.tile_pool(name="qa", bufs=1))      # xmq: ->B4
    yap = es.enter_context(tc.tile_pool(name="yap", bufs=1))    # yall: ->B3
    xmq = [qa.tile([128, C], dt, tag=f"xmq{i}", name=f"xmq{i}")
           for i in range(NDT)]
    yall = [yap.tile([128, C], dt, tag=f"yall{i}", name=f"yall{i}")
            for i in range(NDT)]
    with tc.tile_pool(name="qrop", bufs=1) as qrp:
        qro = [qrp.tile([64, C], BF16, tag=f"qro{h}", name=f"qro{h}")
               for h in range(NH)]
        with tc.tile_pool(name="qt", bufs=1) as qt, \
             tc.tile_pool(name="qrot", bufs=2) as qr2, \
             tc.tile_pool(name="qP", bufs=2, space="PSUM") as qp:
            for i in range(NDT):
                xq = qr2.tile([128, C], dt, tag="xq", name="xq", bufs=2)
                sy.dma_start(xq[:], d_in["xqT"].ap()[128 * i:128 * (i + 1), :])
                x0q = qr2.tile([128, C], dt, tag="x0q", name="x0q", bufs=2)
                sy.dma_start(x0q[:],
                             d_in["x0qT"].ap()[128 * i:128 * (i + 1), :])
                ve.tensor_scalar_mul(x0q[:], x0q[:], vecs["rm1v"][:, i:i + 1])
                ve.scalar_tensor_tensor(xmq[i][:], xq[:],
                                        vecs["rm0v"][:, i:i + 1],
                                        x0q[:], ALU.mult, ALU.add)
            nq = rms_norm_T(qt, qr2, qp, xmq, C, "nq")
            # q^T = cq_w @ n^T, streamed in column halves of cq_wT
            for half in range(2):
                cqh = []
                for i in range(NDT):
                    t = qt.tile([128, 512], dt, tag=f"cqh{i}",
                                name=f"cqh{i}")
                    sy.dma_start(t[:], d_in["cq_wT"].ap()
                                 [128 * i:128 * (i + 1),
                                  512 * half:512 * (half + 1)])
                    cqh.append(t)
                for mm in range(4):
                    m = 4 * half + mm
                    qhat = qr2.tile([128, C], dt, tag="qhat", name="qhat",
                                    bufs=2)
                    for (s, w) in _chunks(C):
                        psq = qp.tile([128, w], dt, tag="psq", name="psq",
                                      bufs=2)
                        for i in range(NDT):
                            pe.matmul(psq[:],
                                      cqh[i][:, 128 * mm:128 * (mm + 1)],
                                      nq[i][:, s:s + w],
                                      start=(i == 0), stop=(i == NDT - 1))
                        head_norm(qr2, qp, psq, w,
                                  (qg8T[0:1, 2 * m:2 * m + 1],
                                   qg8T[0:1, 2 * m + 1:2 * m + 2]),
                                  qhat[:, s:s + w])
                    rope(qr2, qhat, tbl["cosq2"], tbl["sinq2"], C,
                         out_pair=(qro[2 * m], qro[2 * m + 1]))

        if d_dbg:
            for h in range(NH):
                sy.dma_start(d_dbg["dbg_qro"].ap()[64 * h:64 * (h + 1), :],
                             qro[h][:])
        # ========================= Stage B2: attention =========================
        with tc.tile_pool(name="at", bufs=1) as at, \
             tc.tile_pool(name="atP", bufs=2, space="PSUM") as atp:
            for b in range(B):
                all_js = [j for g in groups[b] for (j, _, _) in g]
                for kh in range(NKV):
                    kts, vexts = {}, {}
                    for g in groups[b]:
                        for (j, _, _) in g:
                            r = 4 * b + j // 4
                            loc = 128 * (j % 4)
                            kt = at.tile([64, 128], BF16, tag="kt", name="kt",
                                         bufs=34)
                            sy.dma_start(
                                kt[:],
                                agk_out[KVD * r + 64 * kh:
                                        KVD * r + 64 * (kh + 1),
                                        loc:loc + 128])
                            kts[j] = kt
                            vx = at.tile([128, 65], BF16, tag="vx", name="vx",
                                         bufs=34)
                            sy.dma_start(
                                vx[:, 0:64],
                                agv_out[KVBLK * r + loc:KVBLK * r + loc + 128,
                                        64 * kh:64 * (kh + 1)])
                            gp.memset(vx[:, 64:65], 1.0)
                            vexts[j] = vx
                    for hp in range(2):
                        h0 = 4 * kh + 2 * hp
                        pys = [atp.tile([65, CB], dt, tag="py", name="py",
                                        bufs=2) for _ in range(2)]
                        for g in groups[b]:
                            gw = g[-1][1] + g[-1][2]
                            sts = [atp.tile([128, gw], dt, tag="st",
                                            name="st", bufs=2,
                                            padded_shape=[128, STRIP_MAX])
                                   for _ in range(2)]
                            prb = at.tile([128, 2 * gw], BF16, tag="prb",
                                          name="prb", bufs=2,
                                          padded_shape=[128, 2 * STRIP_MAX])
                            for (j, ofs, Nw) in g:
                                W = int(Wt[b, j])
                                for hh in range(2):
                                    qs = qro[h0 + hh][:,
                                             b * CB + W:b * CB + CB]
                                    p0 = 0
                                    while p0 < Nw:
                                        bend = ((ofs + p0) // 512 + 1) * 512
                                        pw = min(Nw - p0, bend - (ofs + p0))
                                        pe.matmul(
                                            sts[hh][:, ofs + p0:ofs + p0 + pw],
                                            kts[j][:], qs[:, p0:p0 + pw],
                                            start=True, stop=True)
                                        p0 += pw
                            for hh in range(2):
                                sc.activation(prb[:, gw * hh:gw * (hh + 1)],
                                              sts[hh][:], ACT.Exp)
                            for (j, ofs, Nw) in g:
                                if (b, j) not in mofs:
                                    continue
                                mo, mw = mofs[(b, j)]
                                mw = min(mw, Nw)
                                for hh in range(2):
                                    o2 = gw * hh + ofs
                                    ve.tensor_mul(prb[:, o2:o2 + mw],
                                                  prb[:, o2:o2 + mw],
                                                  mask_sb[:, mo:mo + mw])
                            for (j, ofs, Nw) in g:
                                W = int(Wt[b, j])
                                for hh in range(2):
                                    pe.matmul(
                                        pys[hh][:, W:CB], vexts[j][:],
                                        prb[:, gw * hh + ofs:
                                            gw * hh + ofs + Nw],
                                        start=(j == all_js[0]),
                                        stop=(j == all_js[-1]),
                                        skip_group_check=True)
                        for hh in range(2):
                            h = h0 + hh
                            rc = at.tile([1, CB], dt, tag="rc", name="rc",
                                         bufs=2)
                            ve.reciprocal(rc[:], pys[hh][64:65, :])
                            yb = at.tile([64, CB], dt, tag="yb", name="yb",
                                         bufs=2)
                            gp.partition_broadcast(yb[:], rc[0:1, :])
                            ve.tensor_mul(
                                yall[h // 2][64 * (h % 2):64 * (h % 2) + 64,
                                             b * CB:b * CB + CB],
                                pys[hh][0:64, :], yb[:])

    if d_dbg:
        for i in range(NDT):
            sy.dma_start(d_dbg["dbg_yall"].ap()[128 * i:128 * (i + 1), :],
                         yall[i][:])
    # ===================== Stage B3: out-proj + PID =====================
    with tc.tile_pool(name="pj", bufs=1) as pj, \
         tc.tile_pool(name="pjR", bufs=2) as pjr, \
         tc.tile_pool(name="pjP", bufs=2, space="PSUM") as pjp:
        for half in range(2):
            pjh = []
            for i in range(NDT):
                t = pj.tile([128, 512], dt, tag=f"pjh{i}", name=f"pjh{i}")
                sy.dma_start(t[:], d_in["proj_wT"].ap()
                             [128 * i:128 * (i + 1),
                              512 * half:512 * (half + 1)])
                pjh.append(t)
            for mm in range(4):
                m = 4 * half + mm
                velm = pjr.tile([128, C], dt, tag="velm", name="velm",
                                bufs=2)
                sy.dma_start(velm[:],
                             d_in["velqT"].ap()[128 * m:128 * (m + 1), :])
                for (s, w) in _chunks(C):
                    pso = pjp.tile([128, w], dt, tag="pso", name="pso",
                                   bufs=2)
                    for i in range(NDT):
                        pe.matmul(pso[:], pjh[i][:, 128 * mm:128 * (mm + 1)],
                                  yall[i][:, s:s + w],
                                  start=(i == 0), stop=(i == NDT - 1))
                    ve.scalar_tensor_tensor(
                        xmq[m][:, s:s + w], pso[:],
                        vecs["ascalev"][:, m:m + 1],
                        xmq[m][:, s:s + w], ALU.mult, ALU.add)
                t2 = pjr.tile([128, C], dt, tag="t2", name="t2", bufs=2)
                ve.tensor_scalar(t2[:], xmq[m][:], vecs["mucv"][:, m:m + 1],
                                 0.3, ALU.subtract, ALU.mult)
                vn = pjr.tile([128, C], dt, tag="vn", name="vn", bufs=2)
                ve.scalar_tensor_tensor(vn[:], velm[:], 0.95, t2[:],
                                        ALU.mult, ALU.subtract)
                ve.tensor_scalar(vn[:], vn[:], 3.0, -3.0, ALU.min, ALU.max)
                sy.dma_start(d_vn.ap()[128 * m:128 * (m + 1), :], vn[:])
                ve.scalar_tensor_tensor(xmq[m][:], vn[:], 0.1 * 0.1,
                                        xmq[m][:], ALU.mult, ALU.add)

    # ============================ Stage B4: MoE ============================
    with tc.tile_pool(name="mo", bufs=1) as mo, \
         tc.tile_pool(name="moR", bufs=2) as mor, \
         tc.tile_pool(name="moP", bufs=2, space="PSUM") as mop:
        if d_dbg:
            for i in range(NDT):
                sy.dma_start(d_dbg["dbg_x2"].ap()[128 * i:128 * (i + 1), :],
                             xmq[i][:])
        mn = rms_norm_T(mo, mor, mop, xmq, C, "mn")
        if d_dbg:
            for i in range(NDT):
                sy.dma_start(d_dbg["dbg_mn"].ap()[128 * i:128 * (i + 1), :],
                             mn[i][:])
        sg, hh_t = [], []
        for half in range(2):
            guh = []
            for i in range(NDT):
                t = mo.tile([128, 512], dt, tag=f"guh{i}", name=f"guh{i}",
                            bufs=2)
                sy.dma_start(t[:], d_in["gu"].ap()
                             [128 * i:128 * (i + 1),
                              512 * half:512 * (half + 1)])
                guh.append(t)
            for mm in range(4):
                m = 4 * half + mm
                for (s, w) in _chunks(C):
                    psh = mop.tile([128, w], dt, tag="psh", name="psh",
                                   bufs=2)
                    for i in range(NDT):
                        pe.matmul(psh[:], guh[i][:, 128 * mm:128 * (mm + 1)],
                                  mn[i][:, s:s + w],
                                  start=(i == 0), stop=(i == NDT - 1))
                    if m < 4:
                        if s == 0:
                            sgm = mo.tile([128, C], dt, tag=f"sg{m}",
                                          name=f"sg{m}")
                            sg.append(sgm)
                        # silu(g) = g * sigmoid(g)
                        sc.activation(sg[m][:, s:s + w], psh[:], ACT.Sigmoid)
                        ve.tensor_mul(sg[m][:, s:s + w], sg[m][:, s:s + w],
                                      psh[:])
                    else:
                        if s == 0:
                            hm = mo.tile([128, C], dt, tag=f"hh{m - 4}",
                                         name=f"hh{m - 4}")
                            hh_t.append(hm)
                        ve.tensor_mul(hh_t[m - 4][:, s:s + w],
                                      sg[m - 4][:, s:s + w], psh[:])
        dnw = []
        for i2 in range(4):
            t = mo.tile([128, D], dt, tag=f"dnw{i2}", name=f"dnw{i2}")
            sy.dma_start(t[:], d_in["dn"].ap()[128 * i2:128 * (i2 + 1), :])
            dnw.append(t)
        for m in range(NDT):
            xo = mor.tile([128, C], dt, tag="xo", name="xo", bufs=2)
            for (s, w) in _chunks(C):
                psm = mop.tile([128, w], dt, tag="psm", name="psm", bufs=2)
                for i2 in range(4):
                    pe.matmul(psm[:], dnw[i2][:, 128 * m:128 * (m + 1)],
                              hh_t[i2][:, s:s + w],
                              start=(i2 == 0), stop=(i2 == 3))
                ve.scalar_tensor_tensor(xo[:, s:s + w], psm[:],
                                        vecs["mscalev"][:, m:m + 1],
                                        xmq[m][:, s:s + w],
                                        ALU.mult, ALU.add)
            sy.dma_start(d_xout.ap()[128 * m:128 * (m + 1), :], xo[:])

    es.close()


# revision 25
# speedup vs baseline: 1.5980x; 1.1372x over previous
"""Trainium2 Bass kernel for nn_Block_85598698209846 (moe_routing).

Strategy (8 NeuronCores, SPMD single program, per-core data):
- Tokens are assigned to cores BY EXPERT (host routes via eids): core c owns
  exactly the tokens that route to expert c, sorted by (batch, position).
  MoE then needs no communication and each core loads only its expert.
- Attention: K/V are computed in contiguous position blocks (core r owns
  block r) and shared via one 8-core AllGather; each core computes Q for its
  scattered-but-sorted tokens. Causality is recovered with compile-time
  column windows (shared across cores) plus small per-core uploaded masks.
  Softmax runs without max-subtraction (|scores| <= 8 since q,k are
  RMS-normed and scaled by 1/8), matching the reference exactly.
- Layout: all activations transposed [D on partitions, tokens on free], so
  no on-device transposes anywhere; host pre-transposes weights/slices.
"""
import contextlib
import numpy as np
import ml_dtypes

import concourse.bass as bass
import concourse.bacc as bacc
import concourse.tile as tile
from concourse import mybir
from concourse.bass_utils import run_bass_kernel_spmd

B, S, D = 2, 2048, 1024
NH, NKV, HD = 16, 4, 64
KVD = NKV * HD
NE, INTER = 8, 512
EPS = float(np.float32(1.1920929e-07))
NCORES = 8
KVBLK = 512          # seq rows per core in the KV phase
NKVT = S // 128      # 16 kv tiles per batch
NDT = D // 128       # 8 d-tiles
F32 = mybir.dt.float32
BF16 = mybir.dt.bfloat16
STRIP_MAX = 1536     # max score-strip width (3 PSUM banks)
ALU = mybir.AluOpType
ACT = mybir.ActivationFunctionType


# ---------------------------------------------------------------- host side

def _route(eids):
    eids = np.asarray(eids).astype(np.int64)
    lists = [[np.sort(np.where(eids[b] == e)[0]) for b in range(B)]
             for e in range(NE)]
    maxn = max(len(lists[e][b]) for e in range(NE) for b in range(B))
    CB = max(64, ((maxn + 63) // 64) * 64)
    cols = np.zeros((NE, B, CB), dtype=np.int64)
    nreal = np.zeros((NE, B), dtype=np.int64)
    for e in range(NE):
        for b in range(B):
            L = lists[e][b]
            nreal[e, b] = len(L)
            if len(L):
                cols[e, b, :len(L)] = L
                cols[e, b, len(L):] = L[-1]
    return cols, nreal, CB


def _windows(cols, CB):
    Wt = np.zeros((B, NKVT), dtype=np.int64)
    Mt = np.zeros((B, NKVT), dtype=np.int64)
    for b in range(B):
        for j in range(NKVT):
            Wt[b, j] = min(int(np.searchsorted(cols[e, b], 128 * j))
                           for e in range(NE))
            Mt[b, j] = max(int(np.searchsorted(cols[e, b], 128 * j + 127))
                           for e in range(NE))
    return Wt, Mt


def _strip_groups(Wt, CB):
    """Per batch: greedy-pack kv tiles into strip groups of width <= STRIP_MAX.
    groups[b] = list of groups; each group = list of (j, ofs_in_group, Nw)."""
    groups = []
    for b in range(B):
        gs, cur, ofs = [], [], 0
        for j in range(NKVT):
            Nw = int(CB - Wt[b, j])
            if Nw <= 0:
                continue
            if ofs + Nw > STRIP_MAX:
                gs.append(cur)
                cur, ofs = [], 0
            cur.append((j, ofs, Nw))
            ofs += Nw
        if cur:
            gs.append(cur)
        groups.append(gs)
    return groups


def _mask_layout(Wt, Mt, CB):
    ofs, total = {}, 0
    for b in range(B):
        for j in range(NKVT):
            if Wt[b, j] >= CB:
                continue
            mw = int(min(Mt[b, j], CB) - Wt[b, j])
            if mw <= 0:
                continue
            ofs[(b, j)] = (total, mw)
            total += mw
    return ofs, max(total, 1)


def _rope_tables(positions):
    """[128, n] cos2/sin2 for full-tile rope (2 heads/tile, swap32 form)."""
    inv_freq = (1.0 / 10000.0 ** (np.arange(0, HD, 2, dtype=np.float32) / HD)
                ).astype(np.float32)
    fr = np.outer(positions.astype(np.float32), inv_freq).astype(np.float32)
    c = np.cos(fr).astype(np.float32).T             # [32, n]
    s = np.sin(fr).astype(np.float32).T
    cos2 = np.concatenate([c, c, c, c], axis=0)
    sin2 = np.concatenate([s, -s, s, -s], axis=0)
    return np.ascontiguousarray(cos2), np.ascontiguousarray(sin2)


def _vec8(v):
    return np.ascontiguousarray(np.asarray(v, np.float32).reshape(NDT, 128).T)


def _build_core_inputs(c, x, x0, vel, rm0, rm1, attn_scale, mlp_scale, mu_c,
                       qg8, cq_wT, ck_wT, cv_wT, proj_wT, gate_up, down,
                       cols, CB, Wt, Mt, mofs, MW):
    f = np.float32
    pos = cols[c]                                     # [B, CB]
    bidx = np.repeat(np.arange(B), CB)
    sidx = pos.reshape(-1)
    b_kv, blk = c // 4, c % 4
    rows = slice(KVBLK * blk, KVBLK * blk + KVBLK)
    cosq2, sinq2 = _rope_tables(sidx)
    cosk2, sink2 = _rope_tables(np.arange(KVBLK * blk, KVBLK * blk + KVBLK))
    mask = np.zeros((128, MW), f)
    for (b, j), (o, mw) in mofs.items():
        W = Wt[b, j]
        kvp = np.arange(128 * j, 128 * j + 128)
        mask[:, o:o + mw] = (pos[b, None, W:W + mw] >= kvp[:, None])
    T = lambda a: np.ascontiguousarray(a.T.astype(f))
    return {
        "xqT": T(x[bidx, sidx]), "x0qT": T(x0[bidx, sidx]),
        "velqT": T(vel[bidx, sidx]),
        "xkvT": T(x[b_kv, rows]), "x0kvT": T(x0[b_kv, rows]),
        "cq_wT": cq_wT, "ck_wT": ck_wT, "cv_wT": cv_wT, "proj_wT": proj_wT,
        "gu": np.ascontiguousarray(gate_up[c].astype(f)),
        "dn": np.ascontiguousarray(down[c].astype(f)),
        "rm0v": _vec8(rm0), "rm1v": _vec8(rm1), "ascalev": _vec8(attn_scale),
        "mscalev": _vec8(mlp_scale), "mucv": _vec8(mu_c),
        "qg8T": np.ascontiguousarray(qg8.reshape(1, NH)),
        "cosq2": cosq2, "sinq2": sinq2, "cosk2": cosk2, "sink2": sink2,
        "maskcat": np.ascontiguousarray(mask.astype(ml_dtypes.bfloat16)),
    }


_PROG_CACHE = {}


def _prep(inputs):
    f = np.float32
    x = np.asarray(inputs["x"], f)
    x0 = np.asarray(inputs["x0"], f)
    vel = np.asarray(inputs["vel"], f)
    resid_mix = np.asarray(inputs["resid_mix"], f)
    mu_c = np.clip(np.asarray(inputs["mu"], f), f(0.5), f(1.5)).astype(f)
    qg8 = (np.asarray(inputs["q_gain"], f) * f(0.125)).astype(f)
    cq_wT = np.ascontiguousarray(np.asarray(inputs["cq_w"], f).T)
    ck_wT = np.ascontiguousarray(np.asarray(inputs["ck_w"], f).T)
    cv_wT = np.ascontiguousarray(np.asarray(inputs["cv_w"], f).T)
    proj_wT = np.ascontiguousarray(np.asarray(inputs["proj_w"], f).T)

    cols, nreal, CB = _route(inputs["eids"])
    Wt, Mt = _windows(cols, CB)
    groups = _strip_groups(Wt, CB)
    mofs, MW = _mask_layout(Wt, Mt, CB)
    meta = (cols, nreal, CB, Wt, Mt, groups, mofs, MW)
    in_maps = [
        _build_core_inputs(c, x, x0, vel, resid_mix[0], resid_mix[1],
                           np.asarray(inputs["attn_scale"], f),
                           np.asarray(inputs["mlp_scale"], f), mu_c, qg8,
                           cq_wT, ck_wT, cv_wT, proj_wT,
                           np.asarray(inputs["gate_up"], f),
                           np.asarray(inputs["down"], f),
                           cols, CB, Wt, Mt, mofs, MW)
        for c in range(NCORES)
    ]
    return meta, in_maps


def _assemble(results, meta):
    f = np.float32
    cols, nreal, CB = meta[0], meta[1], meta[2]
    x_out = np.zeros((B, S, D), f)
    v_out = np.zeros((B, S, D), f)
    for c in range(NCORES):
        xoT = results[c]["xoutT"]
        vnT = results[c]["vnT"]
        for b in range(B):
            n = int(nreal[c, b])
            if n == 0:
                continue
            sl = slice(b * CB, b * CB + n)
            x_out[b, cols[c, b, :n]] = xoT[:, sl].T
            v_out[b, cols[c, b, :n]] = vnT[:, sl].T
    return x_out, v_out


def get_program(meta):
    cols, nreal, CB, Wt, Mt, groups, mofs, MW = meta
    key = (CB, MW, tuple(Wt.reshape(-1)), tuple(Mt.reshape(-1)))
    if key not in _PROG_CACHE:
        _PROG_CACHE[key] = build_program(CB, Wt, Mt, groups, mofs, MW)
    return _PROG_CACHE[key]


def kernel(**inputs):
    meta, in_maps = _prep(inputs)
    nc = get_program(meta)
    res = run_bass_kernel_spmd(nc, in_maps, core_ids=list(range(NCORES)))
    return _assemble(res.results, meta)


# ------------------------------------------------------------- device side

def _chunks(n, limit=512):
    return [(s, min(limit, n - s)) for s in range(0, n, limit)]


def build_program(CB, Wt, Mt, groups, mofs, MW, n_devices=NCORES, dbg=False):
    C = B * CB
    nc = bacc.Bacc("TRN2", target_bir_lowering=False, debug=False,
                   num_devices=n_devices)
    dt = F32
    d_in = {}
    for name, shape in [
        ("xqT", [D, C]), ("x0qT", [D, C]), ("velqT", [D, C]),
        ("xkvT", [D, KVBLK]), ("x0kvT", [D, KVBLK]),
        ("cq_wT", [D, D]), ("ck_wT", [D, KVD]), ("cv_wT", [D, KVD]),
        ("proj_wT", [D, D]), ("gu", [D, 2 * INTER]), ("dn", [INTER, D]),
        ("rm0v", [128, NDT]), ("rm1v", [128, NDT]), ("ascalev", [128, NDT]),
        ("mscalev", [128, NDT]), ("mucv", [128, NDT]), ("qg8T", [1, NH]),
        ("cosq2", [128, C]), ("sinq2", [128, C]),
        ("cosk2", [128, KVBLK]), ("sink2", [128, KVBLK]),
    ]:
        d_in[name] = nc.dram_tensor(name, shape, dt, kind="ExternalInput")
    d_in["maskcat"] = nc.dram_tensor("maskcat", [128, MW], BF16,
                                     kind="ExternalInput")
    d_xout = nc.dram_tensor("xoutT", [D, C], dt, kind="ExternalOutput")
    d_vn = nc.dram_tensor("vnT", [D, C], dt, kind="ExternalOutput")
    d_dbg = {}
    if dbg:
        for name, shape in [("dbg_nk", [D, KVBLK]), ("dbg_agk", [2048, 512]),
                            ("dbg_agv", [4096, 256]), ("dbg_qro", [NH * 64, C]),
                            ("dbg_yall", [D, C]), ("dbg_x2", [D, C]),
                            ("dbg_mn", [D, C])]:
            d_dbg[name] = nc.dram_tensor(name, shape, dt,
                                         kind="ExternalOutput")

    with tile.TileContext(nc) as tc:
        _emit(tc, nc, d_in, d_xout, d_vn, CB, Wt, Mt, groups, mofs, d_dbg)
    nc.compile()
    return nc


def _emit(tc, nc, d_in, d_xout, d_vn, CB, Wt, Mt, groups, mofs, d_dbg={}):
    C = B * CB
    dt = F32
    sy, gp, ve, sc, pe = nc.sync, nc.gpsimd, nc.vector, nc.scalar, nc.tensor

    es = contextlib.ExitStack()
    cst = es.enter_context(tc.tile_pool(name="const", bufs=1))
    agd = es.enter_context(tc.tile_pool(name="agD", bufs=1, space="DRAM"))

    ones128 = cst.tile([128, 1], dt, tag="ones128")
    ve.memset(ones128[:], 1.0)
    ind64 = cst.tile([128, 2], dt, tag="ind64")
    ve.memset(ind64[:], 0.0)
    ve.memset(ind64[0:64, 0:1], 1.0)
    ve.memset(ind64[64:128, 1:2], 1.0)
    epsc = cst.tile([128, 1], dt, tag="epsc")
    ve.memset(epsc[:], EPS)
    vecs = {}
    for nm in ("rm0v", "rm1v", "ascalev", "mscalev", "mucv"):
        t = cst.tile([128, NDT], dt, tag=nm, name=nm)
        sy.dma_start(t[:], d_in[nm].ap())
        vecs[nm] = t
    qg8T = cst.tile([1, NH], dt, tag="qg8T")
    sy.dma_start(qg8T[:], d_in["qg8T"].ap())
    tbl = {}
    for nm, w in (("cosq2", C), ("sinq2", C), ("cosk2", KVBLK),
                  ("sink2", KVBLK)):
        t = cst.tile([128, w], dt, tag=nm, name=nm)
        sy.dma_start(t[:], d_in[nm].ap())
        tbl[nm] = t
    mask_sb = cst.tile([128, d_in["maskcat"].shape[1]], BF16, tag="mask")
    sy.dma_start(mask_sb[:], d_in["maskcat"].ap())

    agk_in = agd.tile([KVD, KVBLK], BF16, tag="agk_in")
    agv_in = agd.tile([KVBLK, KVD], BF16, tag="agv_in")
    agk_out = agd.tile([NCORES * KVD, KVBLK], BF16, addr_space="Shared",
                       tag="agk_out")
    agv_out = agd.tile([NCORES * KVBLK, KVD], BF16, addr_space="Shared",
                       tag="agv_out")

    def rms_norm_T(pool, rot, pstmp, in_tiles, width, out_tag):
        outs = [pool.tile([128, width], dt, tag=f"{out_tag}{i}",
                          name=f"{out_tag}{i}") for i in range(NDT)]
        for (s, w) in _chunks(width):
            sqs = []
            for i in range(NDT):
                sq = rot.tile([128, w], dt, tag="nsq", name="nsq", bufs=3)
                ve.tensor_mul(sq[:], in_tiles[i][:, s:s + w],
                              in_tiles[i][:, s:s + w])
                sqs.append(sq)
            ssum = pstmp.tile([1, w], dt, tag="nps", name="nps", bufs=2)
            for i in range(NDT):
                pe.matmul(ssum[:], ones128[:], sqs[i][:],
                          start=(i == 0), stop=(i == NDT - 1))
            rt = rot.tile([1, w], dt, tag="nrt", name="nrt", bufs=2)
            sc.activation(rt[:], ssum[:], ACT.Sqrt, bias=epsc[0:1],
                          scale=1.0 / D)
            rts = rot.tile([1, w], dt, tag="nrts", name="nrts", bufs=2)
            rto = rot.tile([1, w], dt, tag="nrto", name="nrto", bufs=2)
            ve.reciprocal_approx_accurate(rto[:], rt[:], rts[:])
            rt = rto
            bc = rot.tile([128, w], dt, tag="nbc", name="nbc", bufs=2)
            gp.partition_broadcast(bc[:], rt[0:1, :])
            for i in range(NDT):
                ve.tensor_mul(outs[i][:, s:s + w], in_tiles[i][:, s:s + w],
                              bc[:])
        return outs

    def head_norm(rot, pstmp, src_ap, width, gains, out_ap):
        """src [128, width] (2 heads) -> normalized out_ap (SBUF).
        gains: optional pair of [1,1] APs multiplied into the inverses."""
        for (s, w) in _chunks(width):
            sq = rot.tile([128, w], dt, tag="hsq", name="hsq", bufs=2)
            sc.activation(sq[:], src_ap[:, s:s + w], ACT.Square)
            for hh in range(2):
                hs = pstmp.tile([1, w], dt, tag="hps", name="hps", bufs=2)
                pe.matmul(hs[:], ind64[:, hh:hh + 1], sq[:],
                          start=True, stop=True)
                rt = rot.tile([1, w], dt, tag="hrt", name="hrt", bufs=2)
                sc.activation(rt[:], hs[:], ACT.Sqrt, bias=epsc[0:1],
                              scale=1.0 / HD)
                rts = rot.tile([1, w], dt, tag="hrts", name="hrts", bufs=2)
                rto = rot.tile([1, w], dt, tag="hrto", name="hrto", bufs=2)
                ve.reciprocal_approx_accurate(rto[:], rt[:], rts[:])
                rt = rto
                if gains is not None:
                    ve.tensor_scalar_mul(rt[:], rt[:], gains[hh])
                # partition_broadcast only writes base-0 full tiles on HW
                bch = rot.tile([64, w], dt, tag="hbc", name="hbc", bufs=2)
                gp.partition_broadcast(bch[:], rt[0:1, :])
                ve.tensor_mul(out_ap[64 * hh:64 * (hh + 1), s:s + w],
                              src_ap[64 * hh:64 * (hh + 1), s:s + w], bch[:])

    def rope(rot, in_tile, cos2, sin2, width, out_tile=None, out_pair=None):
        sw = rot.tile([128, width], dt, tag="rsw", name="rsw", bufs=2)
        for base in (0, 64):
            ve.tensor_copy(sw[base:base + 32, :],
                           in_tile[base + 32:base + 64, :])
            ve.tensor_copy(sw[base + 32:base + 64, :],
                           in_tile[base:base + 32, :])
        a = rot.tile([128, width], dt, tag="ra", name="ra", bufs=2)
        ve.tensor_mul(a[:], in_tile[:], cos2[:, 0:width])
        ve.tensor_mul(sw[:], sw[:], sin2[:, 0:width])
        if out_pair is not None:
            ve.tensor_add(out_pair[0][:], a[0:64, :], sw[0:64, :])
            ve.tensor_add(out_pair[1][:], a[64:128, :], sw[64:128, :])
        else:
            ve.tensor_add(out_tile[:], a[:], sw[:])

    # ============================ Stage A: KV ============================
    with tc.tile_pool(name="kvA", bufs=1) as kva, \
         tc.tile_pool(name="kvR", bufs=2) as kvr, \
         tc.tile_pool(name="kvP", bufs=2, space="PSUM") as kvp:
        xm = []
        for i in range(NDT):
            xk = kvr.tile([128, KVBLK], dt, tag="xk", name="xk", bufs=2)
            sy.dma_start(xk[:], d_in["xkvT"].ap()[128 * i:128 * (i + 1), :])
            x0k = kvr.tile([128, KVBLK], dt, tag="x0k", name="x0k", bufs=2)
            sy.dma_start(x0k[:], d_in["x0kvT"].ap()[128 * i:128 * (i + 1), :])
            ve.tensor_scalar_mul(x0k[:], x0k[:], vecs["rm1v"][:, i:i + 1])
            t = kva.tile([128, KVBLK], dt, tag=f"xmk{i}", name=f"xmk{i}")
            ve.scalar_tensor_tensor(t[:], xk[:], vecs["rm0v"][:, i:i + 1],
                                    x0k[:], ALU.mult, ALU.add)
            xm.append(t)
        nk = rms_norm_T(kva, kvr, kvp, xm, KVBLK, "nk")
        if d_dbg:
            for i in range(NDT):
                sy.dma_start(d_dbg["dbg_nk"].ap()[128 * i:128 * (i + 1), :],
                             nk[i][:])
        ckw, cvw = [], []
        for i in range(NDT):
            t = kva.tile([128, KVD], dt, tag=f"ckw{i}", name=f"ckw{i}")
            sy.dma_start(t[:], d_in["ck_wT"].ap()[128 * i:128 * (i + 1), :])
            ckw.append(t)
            t2 = kva.tile([128, KVD], dt, tag=f"cvw{i}", name=f"cvw{i}")
            sy.dma_start(t2[:], d_in["cv_wT"].ap()[128 * i:128 * (i + 1), :])
            cvw.append(t2)
        for m in range(2):
            pkT = kvp.tile([128, KVBLK], dt, tag="pkT", name="pkT", bufs=2)
            for i in range(NDT):
                pe.matmul(pkT[:], ckw[i][:, 128 * m:128 * (m + 1)], nk[i][:],
                          start=(i == 0), stop=(i == NDT - 1))
            khat = kvr.tile([128, KVBLK], dt, tag="khat", name="khat", bufs=2)
            head_norm(kvr, kvp, pkT, KVBLK, None, khat)
            kro = kvr.tile([128, KVBLK], BF16, tag="kro", name="kro", bufs=2)
            rope(kvr, khat, tbl["cosk2"], tbl["sink2"], KVBLK, kro)
            sy.dma_start(agk_in[128 * m:128 * (m + 1), :], kro[:])
        for m in range(4):
            pv = kvp.tile([128, KVD], dt, tag="pv", name="pv", bufs=2)
            for i in range(NDT):
                pe.matmul(pv[:], nk[i][:, 128 * m:128 * (m + 1)], cvw[i][:],
                          start=(i == 0), stop=(i == NDT - 1))
            vsb = kvr.tile([128, KVD], BF16, tag="vsb", name="vsb", bufs=2)
            ve.tensor_copy(vsb[:], pv[:])
            sy.dma_start(agv_in[128 * m:128 * (m + 1), :], vsb[:])

    gp.collective_compute("AllGather", ALU.bypass,
                          replica_groups=[list(range(NCORES))],
                          ins=[agk_in.opt()], outs=[agk_out.opt()])
    gp.collective_compute("AllGather", ALU.bypass,
                          replica_groups=[list(range(NCORES))],
                          ins=[agv_in.opt()], outs=[agv_out.opt()])
    if d_dbg:
        sy.dma_start(d_dbg["dbg_agk"].ap(), agk_out[:, :])
        sy.dma_start(d_dbg["dbg_agv"].ap(), agv_out[:, :])

    # ===================== Stage B1: Q mix/norm/proj/rope ====================
    qa = es.enter_context(tc.tile_pool(name="qa", bufs=1))      # xmq: ->B4
    yap = es.enter_context(tc.tile_pool(name="yap", bufs=1))    # yall: ->B3
    xmq = [qa.tile([128, C], dt, tag=f"xmq{i}", name=f"xmq{i}")
           for i in range(NDT)]
    yall = [yap.tile([128, C], dt, tag=f"yall{i}", name=f"yall{i}")
            for i in range(NDT)]
    with tc.tile_pool(name="qrop", bufs=1) as qrp:
        qro = [qrp.tile([64, C], BF16, tag=f"qro{h}", name=f"qro{h}")
               for h in range(NH)]
        with tc.tile_pool(name="qt", bufs=1) as qt, \
             tc.tile_pool(name="qrot", bufs=2) as qr2, \
             tc.tile_pool(name="qP", bufs=2, space="PSUM") as qp:
            for i in range(NDT):
                xq = qr2.tile([128, C], dt, tag="xq", name="xq", bufs=2)
                sy.dma_start(xq[:], d_in["xqT"].ap()[128 * i:128 * (i + 1), :])
                x0q = qr2.tile([128, C], dt, tag="x0q", name="x0q", bufs=2)
                sy.dma_start(x0q[:],
                             d_in["x0qT"].ap()[128 * i:128 * (i + 1), :])
                ve.tensor_scalar_mul(x0q[:], x0q[:], vecs["rm1v"][:, i:i + 1])
                ve.scalar_tensor_tensor(xmq[i][:], xq[:],
                                        vecs["rm0v"][:, i:i + 1],
                                        x0q[:], ALU.mult, ALU.add)
            nq = rms_norm_T(qt, qr2, qp, xmq, C, "nq")
            # q^T = cq_w @ n^T, streamed in column halves of cq_wT
            for half in range(2):
                cqh = []
                for i in range(NDT):
                    t = qt.tile([128, 512], dt, tag=f"cqh{i}",
                                name=f"cqh{i}")
                    sy.dma_start(t[:], d_in["cq_wT"].ap()
                                 [128 * i:128 * (i + 1),
                                  512 * half:512 * (half + 1)])
                    cqh.append(t)
                for mm in range(4):
                    m = 4 * half + mm
                    qhat = qr2.tile([128, C], dt, tag="qhat", name="qhat",
                                    bufs=2)
                    for (s, w) in _chunks(C):
                        psq = qp.tile([128, w], dt, tag="psq", name="psq",
                                      bufs=2)
                        for i in range(NDT):
                            pe.matmul(psq[:],
                                      cqh[i][:, 128 * mm:128 * (mm + 1)],
                                      nq[i][:, s:s + w],
                                      start=(i == 0), stop=(i == NDT - 1))
                        head_norm(qr2, qp, psq, w,
                                  (qg8T[0:1, 2 * m:2 * m + 1],
                                   qg8T[0:1, 2 * m + 1:2 * m + 2]),
                                  qhat[:, s:s + w])
                    rope(qr2, qhat, tbl["cosq2"], tbl["sinq2"], C,
                         out_pair=(qro[2 * m], qro[2 * m + 1]))

        if d_dbg:
            for h in range(NH):
                sy.dma_start(d_dbg["dbg_qro"].ap()[64 * h:64 * (h + 1), :],
                             qro[h][:])
        # ========================= Stage B2: attention =========================
        with tc.tile_pool(name="at", bufs=1) as at, \
             tc.tile_pool(name="atP", bufs=2, space="PSUM") as atp:
            for b in range(B):
                all_js = [j for g in groups[b] for (j, _, _) in g]
                for kh in range(NKV):
                    kts, vexts = {}, {}
                    for g in groups[b]:
                        for (j, _, _) in g:
                            r = 4 * b + j // 4
                            loc = 128 * (j % 4)
                            kt = at.tile([64, 128], BF16, tag="kt", name="kt",
                                         bufs=34)
                            sy.dma_start(
                                kt[:],
                                agk_out[KVD * r + 64 * kh:
                                        KVD * r + 64 * (kh + 1),
                                        loc:loc + 128])
                            kts[j] = kt
                            vx = at.tile([128, 65], BF16, tag="vx", name="vx",
                                         bufs=34)
                            sy.dma_start(
                                vx[:, 0:64],
                                agv_out[KVBLK * r + loc:KVBLK * r + loc + 128,
                                        64 * kh:64 * (kh + 1)])
                            gp.memset(vx[:, 64:65], 1.0)
                            vexts[j] = vx
                    for hp in range(2):
                        h0 = 4 * kh + 2 * hp
                        pys = [atp.tile([65, CB], dt, tag="py", name="py",
                                        bufs=2) for _ in range(2)]
                        for g in groups[b]:
                            gw = g[-1][1] + g[-1][2]
                            sts = [atp.tile([128, gw], dt, tag="st",
                                            name="st", bufs=2,
                                            padded_shape=[128, STRIP_MAX])
                                   for _ in range(2)]
                            prb = at.tile([128, 2 * gw], BF16, tag="prb",
                                          name="prb", bufs=2,
                                          padded_shape=[128, 2 * STRIP_MAX])
                            for (j, ofs, Nw) in g:
                                W = int(Wt[b, j])
                                for hh in range(2):
                                    qs = qro[h0 + hh][:,
                                             b * CB + W:b * CB + CB]
                                    p0 = 0
                                    while p0 < Nw:
                                        bend = ((ofs + p0) // 512 + 1) * 512
                                        pw = min(Nw - p0, bend - (ofs + p0))
                                        pe.matmul(
                                            sts[hh][:, ofs + p0:ofs + p0 + pw],
                                            kts[j][:], qs[:, p0:p0 + pw],
                                            start=True, stop=True)
                                        p0 += pw
                            for hh in range(2):
                                sc.activation(prb[:, gw * hh:gw * (hh + 1)],
                                              sts[hh][:], ACT.Exp)
                            for (j, ofs, Nw) in g:
                                if (b, j) not in mofs:
                                    continue
                                mo, mw = mofs[(b, j)]
                                mw = min(mw, Nw)
                                for hh in range(2):
                                    o2 = gw * hh + ofs
                                    ve.tensor_mul(prb[:, o2:o2 + mw],
                                                  prb[:, o2:o2 + mw],
                                                  mask_sb[:, mo:mo + mw])
                            for (j, ofs, Nw) in g:
                                W = int(Wt[b, j])
                                for hh in range(2):
                                    pe.matmul(
                                        pys[hh][:, W:CB], vexts[j][:],
                                        prb[:, gw * hh + ofs:
                                            gw * hh + ofs + Nw],
                                        start=(j == all_js[0]),
                                        stop=(j == all_js[-1]),
                                        skip_group_check=True)
                        for hh in range(2):
                            h = h0 + hh
                            rc0 = at.tile([1, CB], dt, tag="rc0",
                                          name="rc0", bufs=2)
                            ve.tensor_copy(rc0[:], pys[hh][64:65, :])
                            rcs = at.tile([1, CB], dt, tag="rcs", name="rcs",
                                          bufs=2)
                            rc = at.tile([1, CB], dt, tag="rc", name="rc",
                                         bufs=2)
                            ve.reciprocal_approx_accurate(rc[:], rc0[:],
                                                          rcs[:])
                            yb = at.tile([64, CB], dt, tag="yb", name="yb",
                                         bufs=2)
                            gp.partition_broadcast(yb[:], rc[0:1, :])
                            ve.tensor_mul(
                                yall[h // 2][64 * (h % 2):64 * (h % 2) + 64,
                                             b * CB:b * CB + CB],
                                pys[hh][0:64, :], yb[:])

    if d_dbg:
        for i in range(NDT):
            sy.dma_start(d_dbg["dbg_yall"].ap()[128 * i:128 * (i + 1), :],
                         yall[i][:])
    # ===================== Stage B3: out-proj + PID =====================
    with tc.tile_pool(name="pj", bufs=1) as pj, \
         tc.tile_pool(name="pjR", bufs=2) as pjr, \
         tc.tile_pool(name="pjP", bufs=2, space="PSUM") as pjp:
        for half in range(2):
            pjh = []
            for i in range(NDT):
                t = pj.tile([128, 512], dt, tag=f"pjh{i}", name=f"pjh{i}")
                sy.dma_start(t[:], d_in["proj_wT"].ap()
                             [128 * i:128 * (i + 1),
                              512 * half:512 * (half + 1)])
                pjh.append(t)
            for mm in range(4):
                m = 4 * half + mm
                velm = pjr.tile([128, C], dt, tag="velm", name="velm",
                                bufs=2)
                sy.dma_start(velm[:],
                             d_in["velqT"].ap()[128 * m:128 * (m + 1), :])
                for (s, w) in _chunks(C):
                    pso = pjp.tile([128, w], dt, tag="pso", name="pso",
                                   bufs=2)
                    for i in range(NDT):
                        pe.matmul(pso[:], pjh[i][:, 128 * mm:128 * (mm + 1)],
                                  yall[i][:, s:s + w],
                                  start=(i == 0), stop=(i == NDT - 1))
                    ve.scalar_tensor_tensor(
                        xmq[m][:, s:s + w], pso[:],
                        vecs["ascalev"][:, m:m + 1],
                        xmq[m][:, s:s + w], ALU.mult, ALU.add)
                t2 = pjr.tile([128, C], dt, tag="t2", name="t2", bufs=2)
                ve.tensor_scalar(t2[:], xmq[m][:], vecs["mucv"][:, m:m + 1],
                                 0.3, ALU.subtract, ALU.mult)
                vn = pjr.tile([128, C], dt, tag="vn", name="vn", bufs=2)
                ve.scalar_tensor_tensor(vn[:], velm[:], 0.95, t2[:],
                                        ALU.mult, ALU.subtract)
                ve.tensor_scalar(vn[:], vn[:], 3.0, -3.0, ALU.min, ALU.max)
                sy.dma_start(d_vn.ap()[128 * m:128 * (m + 1), :], vn[:])
                ve.scalar_tensor_tensor(xmq[m][:], vn[:], 0.1 * 0.1,
                                        xmq[m][:], ALU.mult, ALU.add)

    # ============================ Stage B4: MoE ============================
    with tc.tile_pool(name="mo", bufs=1) as mo, \
         tc.tile_pool(name="moR", bufs=2) as mor, \
         tc.tile_pool(name="moP", bufs=2, space="PSUM") as mop:
        if d_dbg:
            for i in range(NDT):
                sy.dma_start(d_dbg["dbg_x2"].ap()[128 * i:128 * (i + 1), :],
                             xmq[i][:])
        mn = rms_norm_T(mo, mor, mop, xmq, C, "mn")
        if d_dbg:
            for i in range(NDT):
                sy.dma_start(d_dbg["dbg_mn"].ap()[128 * i:128 * (i + 1), :],
                             mn[i][:])
        sg, hh_t = [], []
        for half in range(2):
            guh = []
            for i in range(NDT):
                t = mo.tile([128, 512], dt, tag=f"guh{i}", name=f"guh{i}",
                            bufs=2)
                sy.dma_start(t[:], d_in["gu"].ap()
                             [128 * i:128 * (i + 1),
                              512 * half:512 * (half + 1)])
                guh.append(t)
            for mm in range(4):
                m = 4 * half + mm
                for (s, w) in _chunks(C):
                    psh = mop.tile([128, w], dt, tag="psh", name="psh",
                                   bufs=2)
                    for i in range(NDT):
                        pe.matmul(psh[:], guh[i][:, 128 * mm:128 * (mm + 1)],
                                  mn[i][:, s:s + w],
                                  start=(i == 0), stop=(i == NDT - 1))
                    if m < 4:
                        if s == 0:
                            sgm = mo.tile([128, C], dt, tag=f"sg{m}",
                                          name=f"sg{m}")
                            sg.append(sgm)
                        # silu(g) = g * sigmoid(g)
                        sc.activation(sg[m][:, s:s + w], psh[:], ACT.Sigmoid)
                        ve.tensor_mul(sg[m][:, s:s + w], sg[m][:, s:s + w],
                                      psh[:])
                    else:
                        if s == 0:
                            hm = mo.tile([128, C], dt, tag=f"hh{m - 4}",
                                         name=f"hh{m - 4}")
                            hh_t.append(hm)
                        ve.tensor_mul(hh_t[m - 4][:, s:s + w],
                                      sg[m - 4][:, s:s + w], psh[:])
        dnw = []
        for i2 in range(4):
            t = mo.tile([128, D], dt, tag=f"dnw{i2}", name=f"dnw{i2}")
            sy.dma_start(t[:], d_in["dn"].ap()[128 * i2:128 * (i2 + 1), :])
            dnw.append(t)
        for m in range(NDT):
            xo = mor.tile([128, C], dt, tag="xo", name="xo", bufs=2)
            for (s, w) in _chunks(C):
                psm = mop.tile([128, w], dt, tag="psm", name="psm", bufs=2)
                for i2 in range(4):
                    pe.matmul(psm[:], dnw[i2][:, 128 * m:128 * (m + 1)],
                              hh_t[i2][:, s:s + w],
                              start=(i2 == 0), stop=(i2 == 3))
                ve.scalar_tensor_tensor(xo[:, s:s + w], psm[:],
                                        vecs["mscalev"][:, m:m + 1],
                                        xmq[m][:, s:s + w],
                                        ALU.mult, ALU.add)
            sy.dma_start(d_xout.ap()[128 * m:128 * (m + 1), :], xo[:])

    es.close()


# revision 26
# speedup vs baseline: 2.1346x; 1.3357x over previous
"""Trainium2 Bass kernel for nn_Block_85598698209846 (moe_routing).

Strategy (8 NeuronCores, SPMD single program, per-core data):
- Tokens are assigned to cores BY EXPERT (host routes via eids): core c owns
  exactly the tokens that route to expert c, sorted by (batch, position).
  MoE then needs no communication and each core loads only its expert.
- Attention: K/V are computed in contiguous position blocks (core r owns
  block r) and shared via one 8-core AllGather; each core computes Q for its
  scattered-but-sorted tokens. Causality is recovered with compile-time
  column windows (shared across cores) plus small per-core uploaded masks.
  Softmax runs without max-subtraction (|scores| <= 8 since q,k are
  RMS-normed and scaled by 1/8), matching the reference exactly.
- Layout: all activations transposed [D on partitions, tokens on free], so
  no on-device transposes anywhere; host pre-transposes weights/slices.
"""
import contextlib
import numpy as np
import ml_dtypes

import concourse.bass as bass
import concourse.bacc as bacc
import concourse.tile as tile
from concourse import mybir
from concourse.bass_utils import run_bass_kernel_spmd

B, S, D = 2, 2048, 1024
NH, NKV, HD = 16, 4, 64
KVD = NKV * HD
NE, INTER = 8, 512
EPS = float(np.float32(1.1920929e-07))
NCORES = 8
KVBLK = 512          # seq rows per core in the KV phase
NKVT = S // 128      # 16 kv tiles per batch
NDT = D // 128       # 8 d-tiles
F32 = mybir.dt.float32
BF16 = mybir.dt.bfloat16
STRIP_MAX = 1536     # max score-strip width (3 PSUM banks)
ALU = mybir.AluOpType
ACT = mybir.ActivationFunctionType


# ---------------------------------------------------------------- host side

def _route(eids):
    eids = np.asarray(eids).astype(np.int64)
    lists = [[np.sort(np.where(eids[b] == e)[0]) for b in range(B)]
             for e in range(NE)]
    maxn = max(len(lists[e][b]) for e in range(NE) for b in range(B))
    CB = max(64, ((maxn + 63) // 64) * 64)
    cols = np.zeros((NE, B, CB), dtype=np.int64)
    nreal = np.zeros((NE, B), dtype=np.int64)
    for e in range(NE):
        for b in range(B):
            L = lists[e][b]
            nreal[e, b] = len(L)
            if len(L):
                cols[e, b, :len(L)] = L
                cols[e, b, len(L):] = L[-1]
    return cols, nreal, CB


def _windows(cols, CB):
    Wt = np.zeros((B, NKVT), dtype=np.int64)
    Mt = np.zeros((B, NKVT), dtype=np.int64)
    for b in range(B):
        for j in range(NKVT):
            Wt[b, j] = min(int(np.searchsorted(cols[e, b], 128 * j))
                           for e in range(NE))
            Mt[b, j] = max(int(np.searchsorted(cols[e, b], 128 * j + 127))
                           for e in range(NE))
    return Wt, Mt


def _strip_groups(Wt, CB):
    """Per batch: greedy-pack kv tiles into strip groups of width <= STRIP_MAX.
    groups[b] = list of groups; each group = list of (j, ofs_in_group, Nw)."""
    groups = []
    for b in range(B):
        gs, cur, ofs = [], [], 0
        for j in range(NKVT):
            Nw = int(CB - Wt[b, j])
            if Nw <= 0:
                continue
            if ofs + Nw > STRIP_MAX:
                gs.append(cur)
                cur, ofs = [], 0
            cur.append((j, ofs, Nw))
            ofs += Nw
        if cur:
            gs.append(cur)
        groups.append(gs)
    return groups


def _mask_layout(Wt, Mt, CB):
    ofs, total = {}, 0
    for b in range(B):
        for j in range(NKVT):
            if Wt[b, j] >= CB:
                continue
            mw = int(min(Mt[b, j], CB) - Wt[b, j])
            if mw <= 0:
                continue
            ofs[(b, j)] = (total, mw)
            total += mw
    return ofs, max(total, 1)


def _rope_tables(positions):
    """[128, n] cos2/sin2 for full-tile rope (2 heads/tile, swap32 form)."""
    inv_freq = (1.0 / 10000.0 ** (np.arange(0, HD, 2, dtype=np.float32) / HD)
                ).astype(np.float32)
    fr = np.outer(positions.astype(np.float32), inv_freq).astype(np.float32)
    c = np.cos(fr).astype(np.float32).T             # [32, n]
    s = np.sin(fr).astype(np.float32).T
    cos2 = np.concatenate([c, c, c, c], axis=0)
    sin2 = np.concatenate([s, -s, s, -s], axis=0)
    return np.ascontiguousarray(cos2), np.ascontiguousarray(sin2)


def _vec8(v):
    return np.ascontiguousarray(np.asarray(v, np.float32).reshape(NDT, 128).T)


def _build_core_inputs(c, x, x0, vel, rm0, rm1, attn_scale, mlp_scale, mu_c,
                       qg8, cq_wT, ck_wT, cv_wT, proj_wT, gate_up, down,
                       cols, CB, Wt, Mt, mofs, MW):
    f = np.float32
    pos = cols[c]                                     # [B, CB]
    bidx = np.repeat(np.arange(B), CB)
    sidx = pos.reshape(-1)
    b_kv, blk = c // 4, c % 4
    rows = slice(KVBLK * blk, KVBLK * blk + KVBLK)
    cosq2, sinq2 = _rope_tables(sidx)
    cosk2, sink2 = _rope_tables(np.arange(KVBLK * blk, KVBLK * blk + KVBLK))
    mask = np.zeros((128, MW), f)
    for (b, j), (o, mw) in mofs.items():
        W = Wt[b, j]
        kvp = np.arange(128 * j, 128 * j + 128)
        mask[:, o:o + mw] = (pos[b, None, W:W + mw] >= kvp[:, None])
    T = lambda a: np.ascontiguousarray(a.T.astype(f))
    return {
        "xqT": T(x[bidx, sidx]), "x0qT": T(x0[bidx, sidx]),
        "velqT": T(vel[bidx, sidx]),
        "xkvT": T(x[b_kv, rows]), "x0kvT": T(x0[b_kv, rows]),
        "cq_wT": cq_wT, "ck_wT": ck_wT, "cv_wT": cv_wT, "proj_wT": proj_wT,
        "gu": np.ascontiguousarray(gate_up[c].astype(ml_dtypes.bfloat16)),
        "dn": np.ascontiguousarray(down[c].astype(ml_dtypes.bfloat16)),
        "rm0v": _vec8(rm0), "rm1v": _vec8(rm1), "ascalev": _vec8(attn_scale),
        "mscalev": _vec8(mlp_scale), "mucv": _vec8(mu_c),
        "qg8T": np.ascontiguousarray(qg8.reshape(1, NH)),
        "cosq2": cosq2, "sinq2": sinq2, "cosk2": cosk2, "sink2": sink2,
        "maskcat": np.ascontiguousarray(mask.astype(ml_dtypes.bfloat16)),
    }


_PROG_CACHE = {}


def _prep(inputs):
    f = np.float32
    x = np.asarray(inputs["x"], f)
    x0 = np.asarray(inputs["x0"], f)
    vel = np.asarray(inputs["vel"], f)
    resid_mix = np.asarray(inputs["resid_mix"], f)
    mu_c = np.clip(np.asarray(inputs["mu"], f), f(0.5), f(1.5)).astype(f)
    qg8 = (np.asarray(inputs["q_gain"], f) * f(0.125)).astype(f)
    bf = ml_dtypes.bfloat16
    cq_wT = np.ascontiguousarray(np.asarray(inputs["cq_w"], f).T.astype(bf))
    ck_wT = np.ascontiguousarray(np.asarray(inputs["ck_w"], f).T.astype(bf))
    cv_wT = np.ascontiguousarray(np.asarray(inputs["cv_w"], f).T.astype(bf))
    proj_wT = np.ascontiguousarray(np.asarray(inputs["proj_w"], f).T.astype(bf))

    cols, nreal, CB = _route(inputs["eids"])
    Wt, Mt = _windows(cols, CB)
    groups = _strip_groups(Wt, CB)
    mofs, MW = _mask_layout(Wt, Mt, CB)
    meta = (cols, nreal, CB, Wt, Mt, groups, mofs, MW)
    in_maps = [
        _build_core_inputs(c, x, x0, vel, resid_mix[0], resid_mix[1],
                           np.asarray(inputs["attn_scale"], f),
                           np.asarray(inputs["mlp_scale"], f), mu_c, qg8,
                           cq_wT, ck_wT, cv_wT, proj_wT,
                           np.asarray(inputs["gate_up"], f),
                           np.asarray(inputs["down"], f),
                           cols, CB, Wt, Mt, mofs, MW)
        for c in range(NCORES)
    ]
    return meta, in_maps


def _assemble(results, meta):
    f = np.float32
    cols, nreal, CB = meta[0], meta[1], meta[2]
    x_out = np.zeros((B, S, D), f)
    v_out = np.zeros((B, S, D), f)
    for c in range(NCORES):
        xoT = results[c]["xoutT"]
        vnT = results[c]["vnT"]
        for b in range(B):
            n = int(nreal[c, b])
            if n == 0:
                continue
            sl = slice(b * CB, b * CB + n)
            x_out[b, cols[c, b, :n]] = xoT[:, sl].T
            v_out[b, cols[c, b, :n]] = vnT[:, sl].T
    return x_out, v_out


def get_program(meta):
    cols, nreal, CB, Wt, Mt, groups, mofs, MW = meta
    key = (CB, MW, tuple(Wt.reshape(-1)), tuple(Mt.reshape(-1)))
    if key not in _PROG_CACHE:
        _PROG_CACHE[key] = build_program(CB, Wt, Mt, groups, mofs, MW)
    return _PROG_CACHE[key]


def kernel(**inputs):
    meta, in_maps = _prep(inputs)
    nc = get_program(meta)
    res = run_bass_kernel_spmd(nc, in_maps, core_ids=list(range(NCORES)))
    return _assemble(res.results, meta)


# ------------------------------------------------------------- device side

def _chunks(n, limit=512):
    return [(s, min(limit, n - s)) for s in range(0, n, limit)]


def build_program(CB, Wt, Mt, groups, mofs, MW, n_devices=NCORES, dbg=False):
    C = B * CB
    nc = bacc.Bacc("TRN2", target_bir_lowering=False, debug=False,
                   num_devices=n_devices)
    dt = F32
    d_in = {}
    for name, shape in [
        ("xqT", [D, C]), ("x0qT", [D, C]), ("velqT", [D, C]),
        ("xkvT", [D, KVBLK]), ("x0kvT", [D, KVBLK]),

        ("rm0v", [128, NDT]), ("rm1v", [128, NDT]), ("ascalev", [128, NDT]),
        ("mscalev", [128, NDT]), ("mucv", [128, NDT]), ("qg8T", [1, NH]),
        ("cosq2", [128, C]), ("sinq2", [128, C]),
        ("cosk2", [128, KVBLK]), ("sink2", [128, KVBLK]),
    ]:
        d_in[name] = nc.dram_tensor(name, shape, dt, kind="ExternalInput")
    d_in["maskcat"] = nc.dram_tensor("maskcat", [128, MW], BF16,
                                     kind="ExternalInput")
    for name, shape in [("cq_wT", [D, D]), ("ck_wT", [D, KVD]),
                        ("cv_wT", [D, KVD]), ("proj_wT", [D, D]),
                        ("gu", [D, 2 * INTER]), ("dn", [INTER, D])]:
        d_in[name] = nc.dram_tensor(name, shape, BF16, kind="ExternalInput")
    d_xout = nc.dram_tensor("xoutT", [D, C], dt, kind="ExternalOutput")
    d_vn = nc.dram_tensor("vnT", [D, C], dt, kind="ExternalOutput")
    d_dbg = {}
    if dbg:
        for name, shape in [("dbg_nk", [D, KVBLK]), ("dbg_agk", [2048, 512]),
                            ("dbg_agv", [4096, 256]), ("dbg_qro", [NH * 64, C]),
                            ("dbg_yall", [D, C]), ("dbg_x2", [D, C]),
                            ("dbg_mn", [D, C])]:
            d_dbg[name] = nc.dram_tensor(name, shape, dt,
                                         kind="ExternalOutput")

    with tile.TileContext(nc) as tc:
        _emit(tc, nc, d_in, d_xout, d_vn, CB, Wt, Mt, groups, mofs, d_dbg)
    nc.compile()
    return nc


def _emit(tc, nc, d_in, d_xout, d_vn, CB, Wt, Mt, groups, mofs, d_dbg={}):
    C = B * CB
    dt = F32
    sy, gp, ve, sc, pe = nc.sync, nc.gpsimd, nc.vector, nc.scalar, nc.tensor

    es = contextlib.ExitStack()
    cst = es.enter_context(tc.tile_pool(name="const", bufs=1))
    agd = es.enter_context(tc.tile_pool(name="agD", bufs=1, space="DRAM"))

    ones128 = cst.tile([128, 1], dt, tag="ones128")
    ve.memset(ones128[:], 1.0)
    ind64 = cst.tile([128, 2], dt, tag="ind64")
    ve.memset(ind64[:], 0.0)
    ve.memset(ind64[0:64, 0:1], 1.0)
    ve.memset(ind64[64:128, 1:2], 1.0)
    epsc = cst.tile([128, 1], dt, tag="epsc")
    ve.memset(epsc[:], EPS)
    vecs = {}
    for nm in ("rm0v", "rm1v", "ascalev", "mscalev", "mucv"):
        t = cst.tile([128, NDT], dt, tag=nm, name=nm)
        sy.dma_start(t[:], d_in[nm].ap())
        vecs[nm] = t
    qg8T = cst.tile([1, NH], dt, tag="qg8T")
    sy.dma_start(qg8T[:], d_in["qg8T"].ap())
    tbl = {}
    for nm, w in (("cosq2", C), ("sinq2", C), ("cosk2", KVBLK),
                  ("sink2", KVBLK)):
        t = cst.tile([128, w], dt, tag=nm, name=nm)
        sy.dma_start(t[:], d_in[nm].ap())
        tbl[nm] = t
    mask_sb = cst.tile([128, d_in["maskcat"].shape[1]], BF16, tag="mask")
    sy.dma_start(mask_sb[:], d_in["maskcat"].ap())

    agk_in = agd.tile([KVD, KVBLK], BF16, tag="agk_in")
    agv_in = agd.tile([KVBLK, KVD], BF16, tag="agv_in")
    agk_out = agd.tile([NCORES * KVD, KVBLK], BF16, addr_space="Shared",
                       tag="agk_out")
    agv_out = agd.tile([NCORES * KVBLK, KVD], BF16, addr_space="Shared",
                       tag="agv_out")

    def rms_norm_T(pool, rot, pstmp, in_tiles, width, out_tag, odt=F32):
        outs = [pool.tile([128, width], odt, tag=f"{out_tag}{i}",
                          name=f"{out_tag}{i}") for i in range(NDT)]
        for (s, w) in _chunks(width):
            sqs = []
            for i in range(NDT):
                sq = rot.tile([128, w], dt, tag="nsq", name="nsq", bufs=3)
                ve.tensor_mul(sq[:], in_tiles[i][:, s:s + w],
                              in_tiles[i][:, s:s + w])
                sqs.append(sq)
            ssum = pstmp.tile([1, w], dt, tag="nps", name="nps", bufs=2)
            for i in range(NDT):
                pe.matmul(ssum[:], ones128[:], sqs[i][:],
                          start=(i == 0), stop=(i == NDT - 1))
            rt = rot.tile([1, w], dt, tag="nrt", name="nrt", bufs=2)
            sc.activation(rt[:], ssum[:], ACT.Sqrt, bias=epsc[0:1],
                          scale=1.0 / D)
            rts = rot.tile([1, w], dt, tag="nrts", name="nrts", bufs=2)
            rto = rot.tile([1, w], dt, tag="nrto", name="nrto", bufs=2)
            ve.reciprocal_approx_accurate(rto[:], rt[:], rts[:])
            rt = rto
            bc = rot.tile([128, w], dt, tag="nbc", name="nbc", bufs=2)
            gp.partition_broadcast(bc[:], rt[0:1, :])
            for i in range(NDT):
                ve.tensor_mul(outs[i][:, s:s + w], in_tiles[i][:, s:s + w],
                              bc[:])
        return outs

    def head_norm(rot, pstmp, src_ap, width, gains, out_ap):
        """src [128, width] (2 heads) -> normalized out_ap (SBUF).
        gains: optional pair of [1,1] APs multiplied into the inverses."""
        for (s, w) in _chunks(width):
            sq = rot.tile([128, w], dt, tag="hsq", name="hsq", bufs=2)
            sc.activation(sq[:], src_ap[:, s:s + w], ACT.Square)
            for hh in range(2):
                hs = pstmp.tile([1, w], dt, tag="hps", name="hps", bufs=2)
                pe.matmul(hs[:], ind64[:, hh:hh + 1], sq[:],
                          start=True, stop=True)
                rt = rot.tile([1, w], dt, tag="hrt", name="hrt", bufs=2)
                sc.activation(rt[:], hs[:], ACT.Sqrt, bias=epsc[0:1],
                              scale=1.0 / HD)
                rts = rot.tile([1, w], dt, tag="hrts", name="hrts", bufs=2)
                rto = rot.tile([1, w], dt, tag="hrto", name="hrto", bufs=2)
                ve.reciprocal_approx_accurate(rto[:], rt[:], rts[:])
                rt = rto
                if gains is not None:
                    ve.tensor_scalar_mul(rt[:], rt[:], gains[hh])
                # partition_broadcast only writes base-0 full tiles on HW
                bch = rot.tile([64, w], dt, tag="hbc", name="hbc", bufs=2)
                gp.partition_broadcast(bch[:], rt[0:1, :])
                ve.tensor_mul(out_ap[64 * hh:64 * (hh + 1), s:s + w],
                              src_ap[64 * hh:64 * (hh + 1), s:s + w], bch[:])

    def rope(rot, in_tile, cos2, sin2, width, out_tile=None, out_pair=None):
        sw = rot.tile([128, width], dt, tag="rsw", name="rsw", bufs=2)
        for base in (0, 64):
            ve.tensor_copy(sw[base:base + 32, :],
                           in_tile[base + 32:base + 64, :])
            ve.tensor_copy(sw[base + 32:base + 64, :],
                           in_tile[base:base + 32, :])
        a = rot.tile([128, width], dt, tag="ra", name="ra", bufs=2)
        ve.tensor_mul(a[:], in_tile[:], cos2[:, 0:width])
        ve.tensor_mul(sw[:], sw[:], sin2[:, 0:width])
        if out_pair is not None:
            ve.tensor_add(out_pair[0][:], a[0:64, :], sw[0:64, :])
            ve.tensor_add(out_pair[1][:], a[64:128, :], sw[64:128, :])
        else:
            ve.tensor_add(out_tile[:], a[:], sw[:])

    # ============================ Stage A: KV ============================
    with tc.tile_pool(name="kvA", bufs=1) as kva, \
         tc.tile_pool(name="kvR", bufs=2) as kvr, \
         tc.tile_pool(name="kvP", bufs=2, space="PSUM") as kvp:
        xm = []
        for i in range(NDT):
            xk = kvr.tile([128, KVBLK], dt, tag="xk", name="xk", bufs=2)
            sy.dma_start(xk[:], d_in["xkvT"].ap()[128 * i:128 * (i + 1), :])
            x0k = kvr.tile([128, KVBLK], dt, tag="x0k", name="x0k", bufs=2)
            sy.dma_start(x0k[:], d_in["x0kvT"].ap()[128 * i:128 * (i + 1), :])
            ve.tensor_scalar_mul(x0k[:], x0k[:], vecs["rm1v"][:, i:i + 1])
            t = kva.tile([128, KVBLK], dt, tag=f"xmk{i}", name=f"xmk{i}")
            ve.scalar_tensor_tensor(t[:], xk[:], vecs["rm0v"][:, i:i + 1],
                                    x0k[:], ALU.mult, ALU.add)
            xm.append(t)
        nk = rms_norm_T(kva, kvr, kvp, xm, KVBLK, "nk", odt=BF16)
        if d_dbg:
            for i in range(NDT):
                sy.dma_start(d_dbg["dbg_nk"].ap()[128 * i:128 * (i + 1), :],
                             nk[i][:])
        ckw, cvw = [], []
        for i in range(NDT):
            t = kva.tile([128, KVD], BF16, tag=f"ckw{i}", name=f"ckw{i}")
            sy.dma_start(t[:], d_in["ck_wT"].ap()[128 * i:128 * (i + 1), :])
            ckw.append(t)
            t2 = kva.tile([128, KVD], BF16, tag=f"cvw{i}", name=f"cvw{i}")
            sy.dma_start(t2[:], d_in["cv_wT"].ap()[128 * i:128 * (i + 1), :])
            cvw.append(t2)
        for m in range(2):
            pkT = kvp.tile([128, KVBLK], dt, tag="pkT", name="pkT", bufs=2)
            for i in range(NDT):
                pe.matmul(pkT[:], ckw[i][:, 128 * m:128 * (m + 1)], nk[i][:],
                          start=(i == 0), stop=(i == NDT - 1))
            khat = kvr.tile([128, KVBLK], dt, tag="khat", name="khat", bufs=2)
            head_norm(kvr, kvp, pkT, KVBLK, None, khat)
            kro = kvr.tile([128, KVBLK], BF16, tag="kro", name="kro", bufs=2)
            rope(kvr, khat, tbl["cosk2"], tbl["sink2"], KVBLK, kro)
            sy.dma_start(agk_in[128 * m:128 * (m + 1), :], kro[:])
        for m in range(4):
            pv = kvp.tile([128, KVD], dt, tag="pv", name="pv", bufs=2)
            for i in range(NDT):
                pe.matmul(pv[:], nk[i][:, 128 * m:128 * (m + 1)], cvw[i][:],
                          start=(i == 0), stop=(i == NDT - 1))
            vsb = kvr.tile([128, KVD], BF16, tag="vsb", name="vsb", bufs=2)
            ve.tensor_copy(vsb[:], pv[:])
            sy.dma_start(agv_in[128 * m:128 * (m + 1), :], vsb[:])

    gp.collective_compute("AllGather", ALU.bypass,
                          replica_groups=[list(range(NCORES))],
                          ins=[agk_in.opt()], outs=[agk_out.opt()])
    gp.collective_compute("AllGather", ALU.bypass,
                          replica_groups=[list(range(NCORES))],
                          ins=[agv_in.opt()], outs=[agv_out.opt()])
    if d_dbg:
        sy.dma_start(d_dbg["dbg_agk"].ap(), agk_out[:, :])
        sy.dma_start(d_dbg["dbg_agv"].ap(), agv_out[:, :])

    # ===================== Stage B1: Q mix/norm/proj/rope ====================
    qa = es.enter_context(tc.tile_pool(name="qa", bufs=1))      # xmq: ->B4
    yap = es.enter_context(tc.tile_pool(name="yap", bufs=1))    # yall: ->B3
    xmq = [qa.tile([128, C], dt, tag=f"xmq{i}", name=f"xmq{i}")
           for i in range(NDT)]
    yall = [yap.tile([128, C], BF16, tag=f"yall{i}", name=f"yall{i}")
            for i in range(NDT)]
    with tc.tile_pool(name="qrop", bufs=1) as qrp:
        qro = [qrp.tile([64, C], BF16, tag=f"qro{h}", name=f"qro{h}")
               for h in range(NH)]
        with tc.tile_pool(name="qt", bufs=1) as qt, \
             tc.tile_pool(name="qrot", bufs=2) as qr2, \
             tc.tile_pool(name="qP", bufs=2, space="PSUM") as qp:
            for i in range(NDT):
                xq = qr2.tile([128, C], dt, tag="xq", name="xq", bufs=2)
                sy.dma_start(xq[:], d_in["xqT"].ap()[128 * i:128 * (i + 1), :])
                x0q = qr2.tile([128, C], dt, tag="x0q", name="x0q", bufs=2)
                sy.dma_start(x0q[:],
                             d_in["x0qT"].ap()[128 * i:128 * (i + 1), :])
                ve.tensor_scalar_mul(x0q[:], x0q[:], vecs["rm1v"][:, i:i + 1])
                ve.scalar_tensor_tensor(xmq[i][:], xq[:],
                                        vecs["rm0v"][:, i:i + 1],
                                        x0q[:], ALU.mult, ALU.add)
            nq = rms_norm_T(qt, qr2, qp, xmq, C, "nq", odt=BF16)
            # q^T = cq_w @ n^T, streamed in column halves of cq_wT
            for half in range(2):
                cqh = []
                for i in range(NDT):
                    t = qt.tile([128, 512], BF16, tag=f"cqh{i}",
                                name=f"cqh{i}")
                    sy.dma_start(t[:], d_in["cq_wT"].ap()
                                 [128 * i:128 * (i + 1),
                                  512 * half:512 * (half + 1)])
                    cqh.append(t)
                for mm in range(4):
                    m = 4 * half + mm
                    qhat = qr2.tile([128, C], dt, tag="qhat", name="qhat",
                                    bufs=2)
                    for (s, w) in _chunks(C):
                        psq = qp.tile([128, w], dt, tag="psq", name="psq",
                                      bufs=2)
                        for i in range(NDT):
                            pe.matmul(psq[:],
                                      cqh[i][:, 128 * mm:128 * (mm + 1)],
                                      nq[i][:, s:s + w],
                                      start=(i == 0), stop=(i == NDT - 1))
                        head_norm(qr2, qp, psq, w,
                                  (qg8T[0:1, 2 * m:2 * m + 1],
                                   qg8T[0:1, 2 * m + 1:2 * m + 2]),
                                  qhat[:, s:s + w])
                    rope(qr2, qhat, tbl["cosq2"], tbl["sinq2"], C,
                         out_pair=(qro[2 * m], qro[2 * m + 1]))

        if d_dbg:
            for h in range(NH):
                sy.dma_start(d_dbg["dbg_qro"].ap()[64 * h:64 * (h + 1), :],
                             qro[h][:])
        # ========================= Stage B2: attention =========================
        with tc.tile_pool(name="at", bufs=1) as at, \
             tc.tile_pool(name="atP", bufs=2, space="PSUM") as atp:
            for b in range(B):
                all_js = [j for g in groups[b] for (j, _, _) in g]
                for kh in range(NKV):
                    kts, vexts = {}, {}
                    for g in groups[b]:
                        for (j, _, _) in g:
                            r = 4 * b + j // 4
                            loc = 128 * (j % 4)
                            kt = at.tile([64, 128], BF16, tag="kt", name="kt",
                                         bufs=34)
                            sy.dma_start(
                                kt[:],
                                agk_out[KVD * r + 64 * kh:
                                        KVD * r + 64 * (kh + 1),
                                        loc:loc + 128])
                            kts[j] = kt
                            vx = at.tile([128, 65], BF16, tag="vx", name="vx",
                                         bufs=34)
                            sy.dma_start(
                                vx[:, 0:64],
                                agv_out[KVBLK * r + loc:KVBLK * r + loc + 128,
                                        64 * kh:64 * (kh + 1)])
                            gp.memset(vx[:, 64:65], 1.0)
                            vexts[j] = vx
                    for hp in range(2):
                        h0 = 4 * kh + 2 * hp
                        pys = [atp.tile([65, CB], dt, tag="py", name="py",
                                        bufs=2) for _ in range(2)]
                        for g in groups[b]:
                            gw = g[-1][1] + g[-1][2]
                            sts = [atp.tile([128, gw], dt, tag="st",
                                            name="st", bufs=2,
                                            padded_shape=[128, STRIP_MAX])
                                   for _ in range(2)]
                            prb = at.tile([128, 2 * gw], BF16, tag="prb",
                                          name="prb", bufs=2,
                                          padded_shape=[128, 2 * STRIP_MAX])
                            for (j, ofs, Nw) in g:
                                W = int(Wt[b, j])
                                for hh in range(2):
                                    qs = qro[h0 + hh][:,
                                             b * CB + W:b * CB + CB]
                                    p0 = 0
                                    while p0 < Nw:
                                        bend = ((ofs + p0) // 512 + 1) * 512
                                        pw = min(Nw - p0, bend - (ofs + p0))
                                        pe.matmul(
                                            sts[hh][:, ofs + p0:ofs + p0 + pw],
                                            kts[j][:], qs[:, p0:p0 + pw],
                                            start=True, stop=True)
                                        p0 += pw
                            for hh in range(2):
                                sc.activation(prb[:, gw * hh:gw * (hh + 1)],
                                              sts[hh][:], ACT.Exp)
                            for (j, ofs, Nw) in g:
                                if (b, j) not in mofs:
                                    continue
                                mo, mw = mofs[(b, j)]
                                mw = min(mw, Nw)
                                for hh in range(2):
                                    o2 = gw * hh + ofs
                                    ve.tensor_mul(prb[:, o2:o2 + mw],
                                                  prb[:, o2:o2 + mw],
                                                  mask_sb[:, mo:mo + mw])
                            for (j, ofs, Nw) in g:
                                W = int(Wt[b, j])
                                for hh in range(2):
                                    pe.matmul(
                                        pys[hh][:, W:CB], vexts[j][:],
                                        prb[:, gw * hh + ofs:
                                            gw * hh + ofs + Nw],
                                        start=(j == all_js[0]),
                                        stop=(j == all_js[-1]),
                                        skip_group_check=True)
                        for hh in range(2):
                            h = h0 + hh
                            rc0 = at.tile([1, CB], dt, tag="rc0",
                                          name="rc0", bufs=2)
                            ve.tensor_copy(rc0[:], pys[hh][64:65, :])
                            rcs = at.tile([1, CB], dt, tag="rcs", name="rcs",
                                          bufs=2)
                            rc = at.tile([1, CB], dt, tag="rc", name="rc",
                                         bufs=2)
                            ve.reciprocal_approx_accurate(rc[:], rc0[:],
                                                          rcs[:])
                            yb = at.tile([64, CB], dt, tag="yb", name="yb",
                                         bufs=2)
                            gp.partition_broadcast(yb[:], rc[0:1, :])
                            ve.tensor_mul(
                                yall[h // 2][64 * (h % 2):64 * (h % 2) + 64,
                                             b * CB:b * CB + CB],
                                pys[hh][0:64, :], yb[:])

    if d_dbg:
        for i in range(NDT):
            sy.dma_start(d_dbg["dbg_yall"].ap()[128 * i:128 * (i + 1), :],
                         yall[i][:])
    # ===================== Stage B3: out-proj + PID =====================
    with tc.tile_pool(name="pj", bufs=1) as pj, \
         tc.tile_pool(name="pjR", bufs=2) as pjr, \
         tc.tile_pool(name="pjP", bufs=2, space="PSUM") as pjp:
        for half in range(2):
            pjh = []
            for i in range(NDT):
                t = pj.tile([128, 512], BF16, tag=f"pjh{i}", name=f"pjh{i}")
                sy.dma_start(t[:], d_in["proj_wT"].ap()
                             [128 * i:128 * (i + 1),
                              512 * half:512 * (half + 1)])
                pjh.append(t)
            for mm in range(4):
                m = 4 * half + mm
                velm = pjr.tile([128, C], dt, tag="velm", name="velm",
                                bufs=2)
                sy.dma_start(velm[:],
                             d_in["velqT"].ap()[128 * m:128 * (m + 1), :])
                for (s, w) in _chunks(C):
                    pso = pjp.tile([128, w], dt, tag="pso", name="pso",
                                   bufs=2)
                    for i in range(NDT):
                        pe.matmul(pso[:], pjh[i][:, 128 * mm:128 * (mm + 1)],
                                  yall[i][:, s:s + w],
                                  start=(i == 0), stop=(i == NDT - 1))
                    ve.scalar_tensor_tensor(
                        xmq[m][:, s:s + w], pso[:],
                        vecs["ascalev"][:, m:m + 1],
                        xmq[m][:, s:s + w], ALU.mult, ALU.add)
                t2 = pjr.tile([128, C], dt, tag="t2", name="t2", bufs=2)
                ve.tensor_scalar(t2[:], xmq[m][:], vecs["mucv"][:, m:m + 1],
                                 0.3, ALU.subtract, ALU.mult)
                vn = pjr.tile([128, C], dt, tag="vn", name="vn", bufs=2)
                ve.scalar_tensor_tensor(vn[:], velm[:], 0.95, t2[:],
                                        ALU.mult, ALU.subtract)
                ve.tensor_scalar(vn[:], vn[:], 3.0, -3.0, ALU.min, ALU.max)
                sy.dma_start(d_vn.ap()[128 * m:128 * (m + 1), :], vn[:])
                ve.scalar_tensor_tensor(xmq[m][:], vn[:], 0.1 * 0.1,
                                        xmq[m][:], ALU.mult, ALU.add)

    # ============================ Stage B4: MoE ============================
    with tc.tile_pool(name="mo", bufs=1) as mo, \
         tc.tile_pool(name="moR", bufs=2) as mor, \
         tc.tile_pool(name="moP", bufs=2, space="PSUM") as mop:
        if d_dbg:
            for i in range(NDT):
                sy.dma_start(d_dbg["dbg_x2"].ap()[128 * i:128 * (i + 1), :],
                             xmq[i][:])
        mn = rms_norm_T(mo, mor, mop, xmq, C, "mn", odt=BF16)
        if d_dbg:
            for i in range(NDT):
                sy.dma_start(d_dbg["dbg_mn"].ap()[128 * i:128 * (i + 1), :],
                             mn[i][:])
        sg, hh_t = [], []
        for half in range(2):
            guh = []
            for i in range(NDT):
                t = mo.tile([128, 512], BF16, tag=f"guh{i}", name=f"guh{i}",
                            bufs=2)
                sy.dma_start(t[:], d_in["gu"].ap()
                             [128 * i:128 * (i + 1),
                              512 * half:512 * (half + 1)])
                guh.append(t)
            for mm in range(4):
                m = 4 * half + mm
                for (s, w) in _chunks(C):
                    psh = mop.tile([128, w], dt, tag="psh", name="psh",
                                   bufs=2)
                    for i in range(NDT):
                        pe.matmul(psh[:], guh[i][:, 128 * mm:128 * (mm + 1)],
                                  mn[i][:, s:s + w],
                                  start=(i == 0), stop=(i == NDT - 1))
                    if m < 4:
                        if s == 0:
                            sgm = mo.tile([128, C], dt, tag=f"sg{m}",
                                          name=f"sg{m}")
                            sg.append(sgm)
                        # silu(g) = g * sigmoid(g)
                        sc.activation(sg[m][:, s:s + w], psh[:], ACT.Sigmoid)
                        ve.tensor_mul(sg[m][:, s:s + w], sg[m][:, s:s + w],
                                      psh[:])
                    else:
                        if s == 0:
                            hm = mo.tile([128, C], BF16, tag=f"hh{m - 4}",
                                         name=f"hh{m - 4}")
                            hh_t.append(hm)
                        ve.tensor_mul(hh_t[m - 4][:, s:s + w],
                                      sg[m - 4][:, s:s + w], psh[:])
        dnw = []
        for i2 in range(4):
            t = mo.tile([128, D], BF16, tag=f"dnw{i2}", name=f"dnw{i2}")
            sy.dma_start(t[:], d_in["dn"].ap()[128 * i2:128 * (i2 + 1), :])
            dnw.append(t)
        for m in range(NDT):
            xo = mor.tile([128, C], dt, tag="xo", name="xo", bufs=2)
            for (s, w) in _chunks(C):
                psm = mop.tile([128, w], dt, tag="psm", name="psm", bufs=2)
                for i2 in range(4):
                    pe.matmul(psm[:], dnw[i2][:, 128 * m:128 * (m + 1)],
                              hh_t[i2][:, s:s + w],
                              start=(i2 == 0), stop=(i2 == 3))
                ve.scalar_tensor_tensor(xo[:, s:s + w], psm[:],
                                        vecs["mscalev"][:, m:m + 1],
                                        xmq[m][:, s:s + w],
                                        ALU.mult, ALU.add)
            sy.dma_start(d_xout.ap()[128 * m:128 * (m + 1), :], xo[:])

    es.close()
